# revision 1
# baseline (speedup 1.0000x reference)
"""MoE-GPT forward on 8 Trainium2 NeuronCores.

Sharding: residual stream replicated on all cores; attention replicated
(transposed-score layout, causal-skipped blocks); MoE expert-parallel
(core c owns expert c, dense over tokens, combine via AllReduce);
lm_head vocab-sharded (8 x 6284 columns), concatenated on host.

Matmuls run in float32r (TF32-like, ~1.5e-4 rel err, 4x fp32 rate);
gating runs in exact fp32 so top-2 routing matches the reference.
"""

import json
import numpy as np

import concourse.bass as bass
import concourse.mybir as mybir
import concourse.tile as tile
from concourse.bass_utils import run_bass_kernel_spmd
from concourse.masks import make_identity

AF = mybir.ActivationFunctionType
ALU = mybir.AluOpType
F32 = mybir.dt.float32
F32R = mybir.dt.float32r
I32 = mybir.dt.int32

L, C, H, E, K, V, T = 2, 768, 12, 8, 2, 50257, 1024
HD = C // H          # 64
F = 4 * C            # 3072
N_CORES = 8
VS = 6284            # vocab shard per core (8*6284 = 50272 >= 50257)
CC = C // 128        # 6 c-chunks
TB = T // 128        # 8 token blocks
FB = F // 128        # 24 f blocks
NEG = -1.0e30


def _legalize_bir_json(bir_bytes):
    """This walrus build accepts at most ONE sync wait per instruction;
    split extras onto standalone NoOps on the same engine."""
    m = json.loads(bir_bytes)
    for f in m["functions"]:
        for bb in f["blocks"]:
            out = []
            for inst in bb["instructions"]:
                si = inst.get("sync_info")
                if si:
                    waits = si.get("on_wait") or []
                    if len(waits) > 1:
                        imm = [w for w in waits if w.get("wait_reg") is None]
                        reg = [w for w in waits if w.get("wait_reg") is not None]
                        keep = reg if reg else [imm[-1]]
                        move = imm if reg else imm[:-1]
                        for j, w in enumerate(move):
                            out.append({
                                "debug": inst.get("debug", 0),
                                "engine": inst["engine"],
                                "ins": [], "outs": [],
                                "name": f"{inst['name']}-lw{j}",
                                "opcode": "NoOp",
                                "sync_info": {"on_wait": [w], "on_update": []},
                            })
                        si["on_wait"] = keep
                out.append(inst)
            bb["instructions"] = out
    return json.dumps(m).encode()


def _ln_apply(nc, pool, out_ap, in_ap, g_ap, eps_tile, rows=128):
    """LayerNorm rows of in_ap [rows, C] -> out_ap, gamma g_ap [rows, C]."""
    stats = pool.tile([128, 3, 6], F32, tag="ln_stats")
    mv = pool.tile([128, 2], F32, tag="ln_mv")
    xg = in_ap.rearrange("p (a b) -> p a b", b=256)
    for sg in range(3):
        nc.vector.bn_stats(out=stats[:rows, sg, :], in_=xg[:, sg, :])
    nc.vector.bn_aggr(out=mv[:rows, :], in_=stats[:rows, :, :])
    mean = mv[:rows, 0:1]
    rstd = pool.tile([128, 1], F32, tag="ln_rstd")
    nc.scalar.activation(out=rstd[:rows, :], in_=mv[:rows, 1:2],
                         func=AF.Sqrt, bias=eps_tile[:rows, :])
    nc.vector.reciprocal(out=rstd[:rows, :], in_=rstd[:rows, :])
    tmp = pool.tile([128, C], F32, tag="ln_tmp")
    nc.vector.tensor_scalar(out=tmp[:rows, :], in0=in_ap,
                            scalar1=mean, scalar2=rstd[:rows, :],
                            op0=ALU.subtract, op1=ALU.mult)
    nc.vector.tensor_tensor(out=out_ap, in0=tmp[:rows, :], in1=g_ap,
                            op=ALU.mult)


def build_program():
    nc = bass.Bass()
    # f32r tiles are deliberate (PE rate); silence the low-precision guard
    nc._allow_low_precision_reason = "f32r matmul inputs are intentional"

    # ---- DRAM parameters ----
    idx = nc.declare_dram_parameter("idx", [1, T], I32, isOutput=False)
    wte = nc.declare_dram_parameter("wte", [V, C], F32, isOutput=False)
    wpe = nc.declare_dram_parameter("wpe", [T, C], F32, isOutput=False)
    qkv_wT = nc.declare_dram_parameter("qkv_wT", [L, C, 3 * C], F32, isOutput=False)
    proj_wT = nc.declare_dram_parameter("proj_wT", [L, C, C], F32, isOutput=False)
    gate_wT = nc.declare_dram_parameter("gate_wT", [L, C, E], F32, isOutput=False)
    ln1_g = nc.declare_dram_parameter("ln1_g", [L, 128, C], F32, isOutput=False)
    ln2_g = nc.declare_dram_parameter("ln2_g", [L, 128, C], F32, isOutput=False)
    lnf_g = nc.declare_dram_parameter("lnf_g", [1, C], F32, isOutput=False)
    evec = nc.declare_dram_parameter("evec", [128, E], F32, isOutput=False)
    w1T = nc.declare_dram_parameter("w1T", [L, C, F], F32, isOutput=False)
    w2T = nc.declare_dram_parameter("w2T", [L, F, C], F32, isOutput=False)
    wteT = nc.declare_dram_parameter("wteT", [C, VS], F32, isOutput=False)
    out = nc.declare_dram_parameter("out", [1, VS], F32, isOutput=True)

    with tile.TileContext(nc) as tc:
        with tc.tile_pool(name="const", bufs=1) as const, \
             tc.tile_pool(name="dram", bufs=1, space="DRAM") as dram, \
             tc.tile_pool(name="xp", bufs=1) as xp, \
             tc.tile_pool(name="small", bufs=2) as small, \
             tc.tile_pool(name="ptrans", bufs=2, space="PSUM") as ptrans, \
             tc.tile_pool(name="psc", bufs=2, space="PSUM") as psc, \
             tc.tile_pool(name="pav", bufs=2, space="PSUM") as pav, \
             tc.tile_pool(name="pbig", bufs=2, space="PSUM") as pbig:

            ident = const.tile([128, 128], F32)
            make_identity(nc, ident)
            eps = const.tile([128, 1], F32)
            nc.vector.memset(eps[:], 1e-5)
            evt = const.tile([128, E], F32)
            nc.sync.dma_start(evt[:], evec[:])
            # causal masks for the 4 diagonal sub-block offsets:
            # mask[rel][p, qf] = 0 if qf - rel*128 - p >= 0 else -1e30
            ones64f = const.tile([1, HD], F32)
            nc.vector.memset(ones64f[:], 1.0)
            ones64 = const.tile([1, HD], F32R)
            nc.scalar.activation(out=ones64[:], in_=ones64f[:], func=AF.Copy)
            onesh = const.tile([128, H], F32)
            nc.vector.memset(onesh[:], 1.0)
            cmask = const.tile([128, 4, 512], F32)
            nc.vector.memset(cmask[:], 0.0)
            for rel in range(4):
                nc.gpsimd.affine_select(
                    out=cmask[:, rel, :], in_=cmask[:, rel, :],
                    pattern=[[1, 512]], base=-rel * 128,
                    channel_multiplier=-1,
                    compare_op=ALU.is_ge, fill=NEG)

            # Residual stream, replicated: X[p, tb, c], token = tb*128+p
            X = xp.tile([128, TB, C], F32)

            # ---- embedding: X = wte[idx] + wpe ----
            for tb in range(TB):
                it = small.tile([128, 1], I32, tag="idx")
                nc.sync.dma_start(it[:], idx[0:1, tb * 128:(tb + 1) * 128]
                                  .rearrange("a b -> b a"))
                emb = small.tile([128, C], F32, tag="emb")
                nc.gpsimd.indirect_dma_start(
                    out=emb[:], out_offset=None, in_=wte[:, :],
                    in_offset=bass.IndirectOffsetOnAxis(ap=it[:, :1], axis=0))
                pe = small.tile([128, C], F32, tag="pe")
                nc.sync.dma_start(pe[:], wpe[tb * 128:(tb + 1) * 128, :])
                nc.vector.tensor_add(out=X[:, tb, :], in0=emb[:], in1=pe[:])

            ar_in = dram.tile([T, C], F32)
            ar_out = dram.tile([T, C], F32)

            for l in range(L):
                g1 = const.tile([128, C], F32, tag="g1", bufs=1)
                nc.sync.dma_start(g1[:], ln1_g[l])
                g2 = const.tile([128, C], F32, tag="g2", bufs=1)
                nc.sync.dma_start(g2[:], ln2_g[l])

                # ======== attention ========
                with tc.tile_pool(name=f"attn{l}", bufs=1) as ap:
                    qT = ap.tile([128, CC, T], F32R)     # scaled by 1/8
                    kT = ap.tile([128, CC, T], F32R)
                    vplus = ap.tile([128, TB, H, HD + 1], F32R)

                    with tc.tile_pool(name=f"aT{l}", bufs=1) as apT, \
                         tc.tile_pool(name=f"attwA{l}", bufs=4) as aw, \
                         tc.tile_pool(name=f"atmpA{l}", bufs=2) as at:
                        aT = apT.tile([128, CC, T], F32R)     # ln1(x)^T

                        # ln1 + transpose -> aT
                        for tb in range(TB):
                            a = at.tile([128, C], F32, tag="lnout")
                            _ln_apply(nc, at, a[:], X[:, tb, :], g1[:], eps)
                            for cc in range(CC):
                                pt = ptrans.tile([128, 128], F32)
                                nc.tensor.transpose(out=pt[:],
                                                    in_=a[:, cc * 128:(cc + 1) * 128],
                                                    identity=ident[:])
                                nc.scalar.activation(out=aT[:, cc, tb * 128:(tb + 1) * 128],
                                                     in_=pt[:], func=AF.Copy)

                        # qT (scaled 1/8), kT : [d, t]
                        for half, dst, scl in ((0, qT, 0.125), (1, kT, 1.0)):
                            for db in range(CC):
                                for tch in range(2):
                                    ps = psc.tile([128, 512], F32, tag="ps")
                                    for cc in range(CC):
                                        wt_ = aw.tile([128, 128], F32R, tag="wqkv")
                                        nc.gpsimd.dma_start(
                                            wt_[:], qkv_wT[l][cc * 128:(cc + 1) * 128,
                                                              half * C + db * 128:
                                                              half * C + (db + 1) * 128])
                                        nc.tensor.matmul(ps[:], wt_[:],
                                                         aT[:, cc, tch * 512:(tch + 1) * 512],
                                                         start=(cc == 0), stop=(cc == CC - 1))
                                    nc.scalar.activation(
                                        out=dst[:, db, tch * 512:(tch + 1) * 512],
                                        in_=ps[:], func=AF.Copy, scale=scl)

                        # v natural -> vplus[:, tb, h, 0:64]; ones in col 64
                        for tb in range(TB):
                            nc.scalar.activation(out=vplus[:, tb, :, HD],
                                                 in_=onesh[:], func=AF.Copy)
                        for tb in range(TB):
                            for nch in range(2):   # 6 heads per chunk of 384
                                pv = psc.tile([128, 384], F32, tag="ps")
                                for cc in range(CC):
                                    wt_ = aw.tile([128, 384], F32R, tag="wv")
                                    nc.gpsimd.dma_start(
                                        wt_[:], qkv_wT[l][cc * 128:(cc + 1) * 128,
                                                          2 * C + nch * 384:
                                                          2 * C + (nch + 1) * 384])
                                    nc.tensor.matmul(pv[:], aT[:, cc, tb * 128:(tb + 1) * 128],
                                                     wt_[:],
                                                     start=(cc == 0), stop=(cc == CC - 1))
                                dstv = vplus[:, tb, nch * 6:(nch + 1) * 6, 0:HD]
                                nc.vector.tensor_copy(out=dstv, in_=pv[:].rearrange(
                                    "p (a b) -> p a b", b=HD))

                    with tc.tile_pool(name=f"attB{l}", bufs=1) as bp, \
                         tc.tile_pool(name=f"attwB{l}", bufs=4) as bw, \
                         tc.tile_pool(name=f"atmpB{l}", bufs=2) as bt:
                        attT = bp.tile([128, CC, T], F32R)

                        # scores^T + exp + av^T per head
                        for h in range(H):
                            hc, hp = h // 2, (h % 2) * HD
                            for qc in range(2):
                                pa = pav.tile([HD + 1, 512], F32, tag="pv")
                                nkb = 4 * (qc + 1)
                                for kb in range(nkb):
                                    ps = psc.tile([128, 512], F32, tag="ps")
                                    nc.tensor.matmul(
                                        ps[:], kT[hp:hp + HD, hc, kb * 128:(kb + 1) * 128],
                                        qT[hp:hp + HD, hc, qc * 512:(qc + 1) * 512],
                                        start=True, stop=True)
                                    es = bt.tile([128, 512], F32R, tag="es")
                                    if kb >= 4 * qc:  # partial-causal block
                                        ms = bt.tile([128, 512], F32, tag="ms")
                                        nc.vector.tensor_tensor(
                                            out=ms[:], in0=ps[:],
                                            in1=cmask[:, kb - 4 * qc, :], op=ALU.add)
                                        nc.scalar.activation(out=es[:], in_=ms[:],
                                                             func=AF.Exp)
                                    else:
                                        nc.scalar.activation(out=es[:], in_=ps[:],
                                                             func=AF.Exp)
                                    nc.tensor.matmul(pa[:], vplus[:, kb, h, :], es[:],
                                                     start=(kb == 0), stop=(kb == nkb - 1))
                                # normalize columns by row 64 (the exp-sums):
                                # broadcast recip-sums to 64 partitions via K=1 matmul
                                rs = bt.tile([1, 512], F32R, tag="rs")
                                with nc.allow_low_precision(reason="f32r for PE bcast"):
                                    nc.vector.reciprocal(out=rs[:], in_=pa[HD:HD + 1, :])
                                pb = psc.tile([128, 512], F32, tag="ps")
                                nc.tensor.matmul(pb[:HD, :], ones64[:], rs[:],
                                                 start=True, stop=True)
                                rsb = bt.tile([HD, 512], F32, tag="rsb")
                                nc.scalar.activation(out=rsb[:], in_=pb[:HD, :],
                                                     func=AF.Copy)
                                nc.vector.tensor_tensor(
                                    out=attT[hp:hp + HD, hc, qc * 512:(qc + 1) * 512],
                                    in0=pa[:HD, :], in1=rsb[:],
                                    op=ALU.mult)

                        # proj + residual (one PSUM tile per 384-col group:
                        # a single matmul output must stay within one bank)
                        for tb in range(TB):
                            for nch in range(2):
                                py = pbig.tile([128, 384], F32, tag="pb")
                                for cc in range(CC):
                                    wt_ = bw.tile([128, 384], F32R, tag="wproj")
                                    nc.gpsimd.dma_start(
                                        wt_[:], proj_wT[l][cc * 128:(cc + 1) * 128,
                                                           nch * 384:(nch + 1) * 384])
                                    nc.tensor.matmul(py[:],
                                                     attT[:, cc, tb * 128:(tb + 1) * 128],
                                                     wt_[:],
                                                     start=(cc == 0), stop=(cc == CC - 1))
                                nc.vector.tensor_add(
                                    out=X[:, tb, nch * 384:(nch + 1) * 384],
                                    in0=X[:, tb, nch * 384:(nch + 1) * 384], in1=py[:])

                # ======== MoE (dense, expert-parallel) ========
                with tc.tile_pool(name=f"moe{l}", bufs=1) as mp, \
                     tc.tile_pool(name=f"mtmp{l}", bufs=2) as mt:
                    aT2r = mp.tile([128, CC, T], F32R)    # rounded, for FFN
                    comb = mp.tile([128, TB], F32)        # my expert's weight

                    with tc.tile_pool(name=f"moeg{l}", bufs=1) as gp:
                        aT2 = gp.tile([128, CC, T], F32)  # exact, for gating
                        for tb in range(TB):
                            a = mt.tile([128, C], F32, tag="lnout2")
                            _ln_apply(nc, mt, a[:], X[:, tb, :], g2[:], eps)
                            for cc in range(CC):
                                pt = ptrans.tile([128, 128], F32)
                                nc.tensor.transpose(out=pt[:],
                                                    in_=a[:, cc * 128:(cc + 1) * 128],
                                                    identity=ident[:])
                                nc.scalar.activation(out=aT2[:, cc, tb * 128:(tb + 1) * 128],
                                                     in_=pt[:], func=AF.Copy)
                                nc.vector.tensor_copy(out=aT2r[:, cc, tb * 128:(tb + 1) * 128],
                                                      in_=pt[:])

                        # gating (exact fp32) + combine weight for my expert
                        gwt = const.tile([128, CC, E], F32, tag="gw", bufs=1)
                        nc.sync.dma_start(gwt[:],
                                          gate_wT[l].rearrange("(a b) e -> b a e", b=128))
                        for tb in range(TB):
                            pg = pav.tile([128, E], F32, tag="pv")
                            for cc in range(CC):
                                nc.tensor.matmul(pg[:], aT2[:, cc, tb * 128:(tb + 1) * 128],
                                                 gwt[:, cc, :],
                                                 start=(cc == 0), stop=(cc == CC - 1))
                            lg = mt.tile([128, E], F32, tag="lg")
                            nc.vector.tensor_copy(out=lg[:], in_=pg[:])
                            m8 = mt.tile([128, 8], F32, tag="m8")
                            nc.vector.max(out=m8[:], in_=lg[:])
                            nv0 = mt.tile([128, 1], F32, tag="nv0")
                            nc.vector.tensor_scalar_mul(out=nv0[:], in0=m8[:, 0:1],
                                                        scalar1=-1.0)
                            el = mt.tile([128, E], F32, tag="el")
                            nc.scalar.activation(out=el[:], in_=lg[:], func=AF.Exp,
                                                 bias=nv0[:])
                            e1 = mt.tile([128, 1], F32, tag="e1")
                            nc.scalar.activation(out=e1[:], in_=m8[:, 1:2], func=AF.Exp,
                                                 bias=nv0[:])
                            nc.vector.tensor_scalar_add(out=e1[:], in0=e1[:], scalar1=1.0)
                            nc.vector.reciprocal(out=e1[:], in_=e1[:])
                            msk = mt.tile([128, E], F32, tag="msk")
                            nc.vector.tensor_scalar(out=msk[:], in0=lg[:],
                                                    scalar1=m8[:, 1:2], scalar2=None,
                                                    op0=ALU.is_ge)
                            nc.vector.tensor_tensor(out=el[:], in0=el[:], in1=msk[:],
                                                    op=ALU.mult)
                            nc.vector.tensor_scalar_mul(out=el[:], in0=el[:], scalar1=e1[:])
                            nc.vector.tensor_tensor(out=el[:], in0=el[:], in1=evt[:],
                                                    op=ALU.mult)
                            nc.vector.reduce_sum(out=comb[:, tb:tb + 1], in_=el[:],
                                                 axis=mybir.AxisListType.X)

                    # FFN in two token halves to keep hT small
                    with tc.tile_pool(name=f"moeh{l}", bufs=1) as hp_, \
                         tc.tile_pool(name=f"moew{l}", bufs=4) as mw:
                        for tcH in range(2):
                            hT = hp_.tile([128, FB, 512], F32R, tag="hT")
                            for fb in range(FB):
                                ph = psc.tile([128, 512], F32, tag="ps")
                                for cc in range(CC):
                                    wt_ = mw.tile([128, 128], F32R, tag="w1t")
                                    nc.gpsimd.dma_start(
                                        wt_[:], w1T[l][cc * 128:(cc + 1) * 128,
                                                       fb * 128:(fb + 1) * 128])
                                    nc.tensor.matmul(ph[:], wt_[:],
                                                     aT2r[:, cc, tcH * 512:(tcH + 1) * 512],
                                                     start=(cc == 0), stop=(cc == CC - 1))
                                nc.scalar.activation(out=hT[:, fb, :],
                                                     in_=ph[:], func=AF.Gelu)

                            # y = hT^T @ w2T, scaled by comb, -> ar_in
                            for tb in range(tcH * 4, tcH * 4 + 4):
                                for nch in range(2):
                                    pyy = pbig.tile([128, 384], F32, tag="pb")
                                    for fb in range(FB):
                                        wt_ = mw.tile([128, 384], F32R, tag="w2t")
                                        nc.gpsimd.dma_start(
                                            wt_[:], w2T[l][fb * 128:(fb + 1) * 128,
                                                           nch * 384:(nch + 1) * 384])
                                        nc.tensor.matmul(
                                            pyy[:],
                                            hT[:, fb, (tb - tcH * 4) * 128:
                                               (tb - tcH * 4 + 1) * 128],
                                            wt_[:],
                                            start=(fb == 0), stop=(fb == FB - 1))
                                    ys = mt.tile([128, 384], F32, tag="ys")
                                    nc.vector.tensor_scalar_mul(out=ys[:], in0=pyy[:],
                                                                scalar1=comb[:, tb:tb + 1])
                                    nc.sync.dma_start(
                                        ar_in[tb * 128:(tb + 1) * 128,
                                              nch * 384:(nch + 1) * 384], ys[:])

                # AllReduce expert contributions; X += sum
                nc.gpsimd.collective_compute(
                    "AllReduce", ALU.add,
                    replica_groups=[list(range(N_CORES))],
                    ins=[ar_in.opt()], outs=[ar_out.opt()])
                for tb in range(TB):
                    mo = small.tile([128, C], F32, tag="mo")
                    nc.sync.dma_start(mo[:], ar_out[tb * 128:(tb + 1) * 128, :])
                    nc.vector.tensor_add(out=X[:, tb, :], in0=X[:, tb, :], in1=mo[:])

            # ======== final LN (last token) + lm_head shard ========
            with tc.tile_pool(name="lmtail", bufs=1) as lt, \
                 tc.tile_pool(name="wld", bufs=4) as wld:
                gf = lt.tile([1, C], F32, tag="gf")
                nc.sync.dma_start(gf[:], lnf_g[:])
                xrow = lt.tile([1, C], F32, tag="xrow")
                nc.sync.dma_start(xrow[:], X[127:128, TB - 1, :])
                xl = lt.tile([1, C], F32, tag="xl")
                _ln_apply(nc, lt, xl[:1, :], xrow[:1, :], gf[:1, :], eps, rows=1)
                xlT = lt.tile([128, CC, 1], F32R, tag="xlT")
                for cc in range(CC):
                    pt = ptrans.tile([128, 128], F32)
                    nc.tensor.transpose(out=pt[:, 0:1],
                                        in_=xl[0:1, cc * 128:(cc + 1) * 128],
                                        identity=ident[0:1, 0:1])
                    nc.scalar.activation(out=xlT[:, cc, 0:1], in_=pt[:, 0:1], func=AF.Copy)
                nvc = VS // 512 + (1 if VS % 512 else 0)   # 13 chunks (last 140)
                lo = lt.tile([1, VS], F32, tag="lo")
                for vc in range(nvc):
                    w = min(512, VS - vc * 512)
                    pl = pav.tile([1, 512], F32, tag="pv")
                    for cc in range(CC):
                        wt_ = wld.tile([128, 512], F32R, tag="wte_t")
                        nc.gpsimd.dma_start(wt_[:, :w],
                                            wteT[cc * 128:(cc + 1) * 128,
                                                 vc * 512:vc * 512 + w])
                        nc.tensor.matmul(pl[:, :w], xlT[:, cc, 0:1], wt_[:, :w],
                                         start=(cc == 0), stop=(cc == CC - 1))
                    nc.vector.tensor_copy(out=lo[:, vc * 512:vc * 512 + w], in_=pl[:, :w])
                nc.sync.dma_start(out[:], lo[:])

    orig = nc.to_json_bytes
    nc.to_json_bytes = lambda: _legalize_bir_json(orig())
    return nc


_NC_CACHE = None


def kernel(**inputs):
    global _NC_CACHE
    idx = np.asarray(inputs["idx"]).astype(np.int32)
    wte = np.ascontiguousarray(np.asarray(inputs["wte"], dtype=np.float32))
    wpe = np.ascontiguousarray(np.asarray(inputs["wpe"], dtype=np.float32))
    ln1_g = np.asarray(inputs["ln1_g"], dtype=np.float32)
    qkv_w = np.asarray(inputs["qkv_w"], dtype=np.float32)
    proj_w = np.asarray(inputs["proj_w"], dtype=np.float32)
    ln2_g = np.asarray(inputs["ln2_g"], dtype=np.float32)
    gate_w = np.asarray(inputs["gate_w"], dtype=np.float32)
    w1 = np.asarray(inputs["w1"], dtype=np.float32)
    w2 = np.asarray(inputs["w2"], dtype=np.float32)
    lnf_g = np.asarray(inputs["lnf_g"], dtype=np.float32)

    qkv_wT = np.ascontiguousarray(qkv_w.transpose(0, 2, 1))
    proj_wT = np.ascontiguousarray(proj_w.transpose(0, 2, 1))
    gate_wT = np.ascontiguousarray(gate_w.transpose(0, 2, 1))
    ln1_rep = np.ascontiguousarray(np.broadcast_to(ln1_g[:, None, :], (L, 128, C)))
    ln2_rep = np.ascontiguousarray(np.broadcast_to(ln2_g[:, None, :], (L, 128, C)))
    wteT_full = np.zeros((C, N_CORES * VS), np.float32)
    wteT_full[:, :V] = wte.T

    if _NC_CACHE is None:
        _NC_CACHE = build_program()
    nc = _NC_CACHE

    in_maps = []
    for c in range(N_CORES):
        ev = np.zeros((128, E), np.float32)
        ev[:, c] = 1.0
        in_maps.append({
            "idx": idx,
            "wte": wte,
            "wpe": wpe,
            "qkv_wT": qkv_wT,
            "proj_wT": proj_wT,
            "gate_wT": gate_wT,
            "ln1_g": ln1_rep,
            "ln2_g": ln2_rep,
            "lnf_g": lnf_g[None, :],
            "evec": ev,
            "w1T": np.ascontiguousarray(w1[:, c].transpose(0, 2, 1)),
            "w2T": np.ascontiguousarray(w2[:, c].transpose(0, 2, 1)),
            "wteT": np.ascontiguousarray(wteT_full[:, c * VS:(c + 1) * VS]),
        })

    res = run_bass_kernel_spmd(nc, in_maps, list(range(N_CORES)))
    kernel.last_result = res
    logits = np.concatenate([res.results[c]["out"][0] for c in range(N_CORES)])
    return logits[:V].reshape(1, 1, V).astype(np.float32)



# revision 12
# speedup vs baseline: 4.8746x; 4.8746x over previous
"""MoE-GPT forward on 8 Trainium2 NeuronCores.

Sharding: residual stream replicated on all cores; attention replicated
(transposed-score layout, causal-skipped blocks); MoE expert-parallel
(core c owns expert c, dense over tokens, combine via AllReduce);
lm_head vocab-sharded (8 x 6284 columns), concatenated on host.

v2: bf16 matmul operands (weights cast on host, activations cast on
device) except the gating path which stays exact fp32 so top-2 routing
matches the reference; all per-layer weights resident in SBUF (no
weight DMA inside matmul loops); bf16 AllReduce; Rsqrt layernorm.
"""

import json
import numpy as np
import ml_dtypes

import concourse.bass as bass
import concourse.mybir as mybir
import concourse.tile as tile
from concourse.bass_utils import run_bass_kernel_spmd
from concourse.masks import make_identity

AF = mybir.ActivationFunctionType
ALU = mybir.AluOpType
F32 = mybir.dt.float32
F32R = mybir.dt.float32r
BF16 = mybir.dt.bfloat16
I32 = mybir.dt.int32

L, C, H, E, K, V, T = 2, 768, 12, 8, 2, 50257, 1024
HD = C // H          # 64
F = 4 * C            # 3072
N_CORES = 8
VS = 6284            # vocab shard per core (8*6284 = 50272 >= 50257)
CC = C // 128        # 6 c-chunks
TB = T // 128        # 8 token blocks
FB = F // 128        # 24 f blocks
NEG = -1.0e30
BFNP = ml_dtypes.bfloat16


def _legalize_bir_json(bir_bytes):
    """This walrus build accepts at most ONE sync wait per instruction;
    split extras onto standalone NoOps on the same engine."""
    m = json.loads(bir_bytes)
    for f in m["functions"]:
        for bb in f["blocks"]:
            out = []
            for inst in bb["instructions"]:
                si = inst.get("sync_info")
                if si:
                    waits = si.get("on_wait") or []
                    if len(waits) > 1:
                        imm = [w for w in waits if w.get("wait_reg") is None]
                        reg = [w for w in waits if w.get("wait_reg") is not None]
                        keep = reg if reg else [imm[-1]]
                        move = imm if reg else imm[:-1]
                        for j, w in enumerate(move):
                            out.append({
                                "debug": inst.get("debug", 0),
                                "engine": inst["engine"],
                                "ins": [], "outs": [],
                                "name": f"{inst['name']}-lw{j}",
                                "opcode": "NoOp",
                                "sync_info": {"on_wait": [w], "on_update": []},
                            })
                        si["on_wait"] = keep
                out.append(inst)
            bb["instructions"] = out
    return json.dumps(m).encode()


def _ln_apply(nc, pool, out_ap, in_ap, g_ap, eps_tile, rows=128):
    """LayerNorm rows of in_ap [rows, C] -> out_ap, gamma g_ap [rows, C]."""
    stats = pool.tile([128, 3, 6], F32, tag="ln_stats")
    mv = pool.tile([128, 2], F32, tag="ln_mv")
    xg = in_ap.rearrange("p (a b) -> p a b", b=256)
    for sg in range(3):
        nc.vector.bn_stats(out=stats[:rows, sg, :], in_=xg[:, sg, :])
    nc.vector.bn_aggr(out=mv[:rows, :], in_=stats[:rows, :, :])
    mean = mv[:rows, 0:1]
    rstd = pool.tile([128, 1], F32, tag="ln_rstd")
    nc.scalar.activation(out=rstd[:rows, :], in_=mv[:rows, 1:2],
                         func=AF.Sqrt, bias=eps_tile[:rows, :])
    nc.vector.reciprocal(out=rstd[:rows, :], in_=rstd[:rows, :])
    tmp = pool.tile([128, C], F32, tag="ln_tmp")
    nc.vector.tensor_scalar(out=tmp[:rows, :], in0=in_ap,
                            scalar1=mean, scalar2=rstd[:rows, :],
                            op0=ALU.subtract, op1=ALU.mult)
    nc.vector.tensor_tensor(out=out_ap, in0=tmp[:rows, :], in1=g_ap,
                            op=ALU.mult)


def build_program():
    nc = bass.Bass()
    nc._allow_low_precision_reason = "bf16/f32r matmul inputs are intentional"

    # ---- DRAM parameters ----
    idx = nc.declare_dram_parameter("idx", [1, T], I32, isOutput=False)
    wte = nc.declare_dram_parameter("wte", [V, C], F32, isOutput=False)
    wpe = nc.declare_dram_parameter("wpe", [T, C], F32, isOutput=False)
    qkv_wT = nc.declare_dram_parameter("qkv_wT", [L, C, 3 * C], BF16, isOutput=False)
    aqkv = nc.declare_dram_parameter("aqkv", [C, 384], BF16, isOutput=False)
    aproj = nc.declare_dram_parameter("aproj", [128, C], BF16, isOutput=False)
    proj_wT = nc.declare_dram_parameter("proj_wT", [L, C, C], BF16, isOutput=False)
    gate_wT = nc.declare_dram_parameter("gate_wT", [L, C, E], F32, isOutput=False)
    ln1_g = nc.declare_dram_parameter("ln1_g", [L, 128, C], F32, isOutput=False)
    ln2_g = nc.declare_dram_parameter("ln2_g", [L, 128, C], F32, isOutput=False)
    lnf_g = nc.declare_dram_parameter("lnf_g", [1, C], F32, isOutput=False)
    evec = nc.declare_dram_parameter("evec", [128, E], F32, isOutput=False)
    w1T = nc.declare_dram_parameter("w1T", [L, C, F], BF16, isOutput=False)
    w2T = nc.declare_dram_parameter("w2T", [L, F, C], BF16, isOutput=False)
    wteT = nc.declare_dram_parameter("wteT", [C, VS], BF16, isOutput=False)
    out = nc.declare_dram_parameter("out", [1, VS], F32, isOutput=True)

    with tile.TileContext(nc) as tc:
        with tc.tile_pool(name="const", bufs=1) as const, \
             tc.tile_pool(name="dram", bufs=1, space="DRAM") as dram, \
             tc.tile_pool(name="xp", bufs=1) as xp, \
             tc.tile_pool(name="small", bufs=2) as small, \
             tc.tile_pool(name="psc", bufs=2, space="PSUM") as psc, \
             tc.tile_pool(name="pav", bufs=2, space="PSUM") as pav, \
             tc.tile_pool(name="pbig", bufs=2, space="PSUM") as pbig:

            ident = const.tile([128, 128], F32)
            make_identity(nc, ident)
            identb = const.tile([128, 128], BF16)
            nc.vector.tensor_copy(out=identb[:], in_=ident[:])
            eps = const.tile([128, 1], F32)
            nc.vector.memset(eps[:], 1e-5)
            evt = const.tile([128, E], F32)
            nc.sync.dma_start(evt[:], evec[:])
            # causal masks for the 4 diagonal sub-block offsets:
            # mask[rel][p, qf] = 0 if qf - rel*128 - p >= 0 else -1e30
            ones64f = const.tile([1, HD], F32)
            nc.vector.memset(ones64f[:], 1.0)
            ones64 = const.tile([1, HD], F32R)
            nc.scalar.activation(out=ones64[:], in_=ones64f[:], func=AF.Copy)
            onesh = const.tile([128, H], BF16)
            nc.vector.memset(onesh[:], 1.0)
            cmask = const.tile([128, 4, 512], F32)
            nc.vector.memset(cmask[:], 0.0)
            for rel in range(4):
                nc.gpsimd.affine_select(
                    out=cmask[:, rel, :], in_=cmask[:, rel, :],
                    pattern=[[1, 512]], base=-rel * 128,
                    channel_multiplier=-1,
                    compare_op=ALU.is_ge, fill=NEG)

            # Residual stream, replicated: X[p, tb, c], token = tb*128+p
            X = xp.tile([128, TB, C], F32)

            # ---- embedding: X = wte[idx] + wpe ----
            for tb in range(TB):
                it = small.tile([128, 1], I32, tag="idx")
                nc.sync.dma_start(it[:], idx[0:1, tb * 128:(tb + 1) * 128]
                                  .rearrange("a b -> b a"))
                emb = small.tile([128, C], F32, tag="emb")
                nc.gpsimd.indirect_dma_start(
                    out=emb[:], out_offset=None, in_=wte[:, :],
                    in_offset=bass.IndirectOffsetOnAxis(ap=it[:, :1], axis=0))
                pe = small.tile([128, C], F32, tag="pe")
                nc.sync.dma_start(pe[:], wpe[tb * 128:(tb + 1) * 128, :])
                nc.gpsimd.tensor_add(out=X[:, tb, :], in0=emb[:], in1=pe[:])

            ar_in = dram.tile([T, C], BF16)
            ar_out = dram.tile([T, C], BF16)

            for l in range(L - 1):
                g1 = const.tile([128, C], F32, tag="g1", bufs=1)
                nc.sync.dma_start(g1[:], ln1_g[l])
                g2 = const.tile([128, C], F32, tag="g2", bufs=1)
                nc.sync.dma_start(g2[:], ln2_g[l])

                # ======== attention (head-sharded: 2 slot-heads per core,
                # partial proj combined via AllReduce) ========
                with tc.tile_pool(name=f"attn{l}", bufs=1) as ap:
                    aqkvw = ap.tile([128, CC, 384], BF16)
                    nc.sync.dma_start(
                        aqkvw[:], aqkv.rearrange("(a p) d -> p a d", p=128))
                    aprojw = ap.tile([128, C], BF16)
                    nc.sync.dma_start(aprojw[:], aproj[:, :])

                    qTs = ap.tile([128, T], BF16)     # scaled by 1/8
                    kTs = ap.tile([128, T], BF16)
                    vplus = ap.tile([128, TB, 2, HD + 1], BF16)

                    with tc.tile_pool(name=f"aT{l}", bufs=1) as apT, \
                         tc.tile_pool(name=f"ptrA{l}", bufs=2, space="PSUM") as ptrA, \
                         tc.tile_pool(name=f"atmpA{l}", bufs=2) as at:
                        aT = apT.tile([128, CC, T], BF16)     # ln1(x)^T

                        # ln1 (bf16 out) + transpose -> aT
                        for tb in range(TB):
                            a = at.tile([128, C], BF16, tag="lnout")
                            _ln_apply(nc, at, a[:], X[:, tb, :], g1[:], eps)
                            for cc in range(CC):
                                pt = ptrA.tile([128, 128], BF16, tag="ptb")
                                nc.tensor.transpose(out=pt[:],
                                                    in_=a[:, cc * 128:(cc + 1) * 128],
                                                    identity=identb[:])
                                nc.scalar.activation(out=aT[:, cc, tb * 128:(tb + 1) * 128],
                                                     in_=pt[:], func=AF.Copy)

                        # my 2 slot-heads' q, k : [128 dims, T]
                        for half, dst, scl in ((0, qTs, 0.125), (1, kTs, 1.0)):
                            for tch in range(2):
                                ps = psc.tile([128, 512], F32, tag="ps")
                                for cc in range(CC):
                                    nc.tensor.matmul(
                                        ps[:],
                                        aqkvw[:, cc, half * 128:(half + 1) * 128],
                                        aT[:, cc, tch * 512:(tch + 1) * 512],
                                        start=(cc == 0), stop=(cc == CC - 1))
                                nc.scalar.activation(
                                    out=dst[:, tch * 512:(tch + 1) * 512],
                                    in_=ps[:], func=AF.Copy, scale=scl)

                        # v token-major for my 2 slot-heads
                        for tb in range(TB):
                            nc.scalar.activation(out=vplus[:, tb, :, HD],
                                                 in_=onesh[:, 0:2], func=AF.Copy)
                        for tb in range(TB):
                            pv = psc.tile([128, 128], F32, tag="ps")
                            for cc in range(CC):
                                nc.tensor.matmul(
                                    pv[:], aT[:, cc, tb * 128:(tb + 1) * 128],
                                    aqkvw[:, cc, 256:384],
                                    start=(cc == 0), stop=(cc == CC - 1))
                            dstv = vplus[:, tb, :, 0:HD]
                            nc.vector.tensor_copy(out=dstv, in_=pv[:].rearrange(
                                "p (a b) -> p a b", b=HD))

                    with tc.tile_pool(name=f"attB{l}", bufs=1) as bp, \
                         tc.tile_pool(name=f"pbigA{l}", bufs=2, space="PSUM") as pbigA, \
                         tc.tile_pool(name=f"atmpB{l}", bufs=2) as bt:
                        attTs = bp.tile([128, T], BF16)

                        # scores^T + exp + av^T; token-half qc outer so the
                        # half's proj partial + AllReduce launch overlap the
                        # next half's score compute
                        for qc in range(2):
                            for slot in range(2):
                                hp = slot * HD
                                pa = pav.tile([HD + 1, 512], F32, tag="pv")
                                nkb = 4 * (qc + 1)
                                for kb in range(nkb):
                                    ps = psc.tile([128, 512], F32, tag="ps")
                                    nc.tensor.matmul(
                                        ps[:], kTs[hp:hp + HD, kb * 128:(kb + 1) * 128],
                                        qTs[hp:hp + HD, qc * 512:(qc + 1) * 512],
                                        start=True, stop=True)
                                    es = bt.tile([128, 512], BF16, tag="es")
                                    if kb >= 4 * qc:  # partial-causal block
                                        ms = bt.tile([128, 512], F32, tag="ms")
                                        nc.vector.tensor_tensor(
                                            out=ms[:], in0=ps[:],
                                            in1=cmask[:, kb - 4 * qc, :], op=ALU.add)
                                        nc.scalar.activation(out=es[:], in_=ms[:],
                                                             func=AF.Exp)
                                    else:
                                        nc.scalar.activation(out=es[:], in_=ps[:],
                                                             func=AF.Exp)
                                    nc.tensor.matmul(pa[:], vplus[:, kb, slot, :], es[:],
                                                     start=(kb == 0), stop=(kb == nkb - 1))
                                sb = bt.tile([1, 512], F32R, tag="sb")
                                with nc.allow_low_precision(reason="f32r for PE bcast"):
                                    nc.scalar.activation(out=sb[:], in_=pa[HD:HD + 1, :],
                                                         func=AF.Copy)
                                pb = psc.tile([128, 512], F32, tag="ps")
                                nc.tensor.matmul(pb[:HD, :], ones64[:], sb[:],
                                                 start=True, stop=True)
                                rsb = bt.tile([HD, 512], F32, tag="rsb")
                                nc.vector.reciprocal(out=rsb[:], in_=pb[:HD, :])
                                nc.vector.tensor_tensor(
                                    out=attTs[hp:hp + HD, qc * 512:(qc + 1) * 512],
                                    in0=pa[:HD, :], in1=rsb[:],
                                    op=ALU.mult)

                            for tq in range(4):
                                tb = qc * 4 + tq
                                for nch in range(2):
                                    py = pbigA.tile([128, 384], F32, tag="pb")
                                    nc.tensor.matmul(
                                        py[:], attTs[:, tb * 128:(tb + 1) * 128],
                                        aprojw[:, nch * 384:(nch + 1) * 384],
                                        start=True, stop=True)
                                    ya = bt.tile([128, 384], BF16, tag="ya")
                                    nc.vector.tensor_copy(out=ya[:], in_=py[:])
                                    nc.sync.dma_start(
                                        arA_in[tb * 128:(tb + 1) * 128,
                                               nch * 384:(nch + 1) * 384], ya[:])
                                if tq % 2 == 1:
                                    ch = (qc * 4 + tq) // 2
                                    nc.gpsimd.collective_compute(
                                        "AllReduce", ALU.add,
                                        replica_groups=[list(range(N_CORES))],
                                        ins=[arA_in[ch * 256:(ch + 1) * 256, :].opt()],
                                        outs=[aroA[ch].opt()])

                # ======== MoE (dense, expert-parallel) ========
                with tc.tile_pool(name=f"moe{l}", bufs=1) as mp, \
                     tc.tile_pool(name=f"mtmp{l}", bufs=2) as mt:
                    # resident FFN weights (bf16)
                    w1sb = mp.tile([128, CC, F], BF16)
                    nc.sync.dma_start(
                        w1sb[:], w1T[l].rearrange("(a p) d -> p a d", p=128))
                    w2sb = mp.tile([128, FB, C], BF16)
                    nc.sync.dma_start(
                        w2sb[:], w2T[l].rearrange("(a p) d -> p a d", p=128))

                    aT2b = mp.tile([128, CC, T], BF16)    # bf16, for FFN
                    comb = mp.tile([128, TB], F32)        # my expert's weight

                    with tc.tile_pool(name=f"moeg{l}", bufs=1) as gp, \
                         tc.tile_pool(name=f"ptrM{l}", bufs=2, space="PSUM") as ptrM:
                        aT2 = gp.tile([128, CC, T], F32)  # exact, for gating
                        for tb in range(TB):
                            if tb % 2 == 0:
                                # that chunk's attention AllReduce has landed;
                                # fold it into X before using those rows
                                ch = tb // 2
                                for tq in range(2):
                                    mo = small.tile([128, C], BF16, tag="mo")
                                    nc.sync.dma_start(
                                        mo[:], aroA[ch][tq * 128:(tq + 1) * 128, :])
                                    nc.vector.tensor_add(out=X[:, tb + tq, :],
                                                         in0=X[:, tb + tq, :],
                                                         in1=mo[:])
                            a = mt.tile([128, C], F32, tag="lnout2")
                            _ln_apply(nc, mt, a[:], X[:, tb, :], g2[:], eps)
                            for cc in range(CC):
                                pt = ptrM.tile([128, 128], F32)
                                nc.tensor.transpose(out=pt[:],
                                                    in_=a[:, cc * 128:(cc + 1) * 128],
                                                    identity=ident[:])
                                nc.scalar.activation(out=aT2[:, cc, tb * 128:(tb + 1) * 128],
                                                     in_=pt[:], func=AF.Copy)
                                nc.vector.tensor_copy(out=aT2b[:, cc, tb * 128:(tb + 1) * 128],
                                                      in_=pt[:])

                        # gating (exact fp32) + combine weight for my expert
                        gwt = const.tile([128, CC, E], F32, tag="gw", bufs=1)
                        nc.sync.dma_start(gwt[:],
                                          gate_wT[l].rearrange("(a b) e -> b a e", b=128))
                        for tb in range(TB):
                            pg = pav.tile([128, E], F32, tag="pv")
                            for cc in range(CC):
                                nc.tensor.matmul(pg[:], aT2[:, cc, tb * 128:(tb + 1) * 128],
                                                 gwt[:, cc, :],
                                                 start=(cc == 0), stop=(cc == CC - 1))
                            lg = mt.tile([128, E], F32, tag="lg")
                            nc.vector.tensor_copy(out=lg[:], in_=pg[:])
                            m8 = mt.tile([128, 8], F32, tag="m8")
                            nc.vector.max(out=m8[:], in_=lg[:])
                            nv0 = mt.tile([128, 1], F32, tag="nv0")
                            nc.vector.tensor_scalar_mul(out=nv0[:], in0=m8[:, 0:1],
                                                        scalar1=-1.0)
                            el = mt.tile([128, E], F32, tag="el")
                            nc.scalar.activation(out=el[:], in_=lg[:], func=AF.Exp,
                                                 bias=nv0[:])
                            e1 = mt.tile([128, 1], F32, tag="e1")
                            nc.scalar.activation(out=e1[:], in_=m8[:, 1:2], func=AF.Exp,
                                                 bias=nv0[:])
                            nc.vector.tensor_scalar_add(out=e1[:], in0=e1[:], scalar1=1.0)
                            nc.vector.reciprocal(out=e1[:], in_=e1[:])
                            msk = mt.tile([128, E], F32, tag="msk")
                            nc.vector.tensor_scalar(out=msk[:], in0=lg[:],
                                                    scalar1=m8[:, 1:2], scalar2=None,
                                                    op0=ALU.is_ge)
                            nc.vector.tensor_tensor(out=el[:], in0=el[:], in1=msk[:],
                                                    op=ALU.mult)
                            nc.vector.tensor_scalar_mul(out=el[:], in0=el[:], scalar1=e1[:])
                            nc.vector.tensor_tensor(out=el[:], in0=el[:], in1=evt[:],
                                                    op=ALU.mult)
                            nc.vector.reduce_sum(out=comb[:, tb:tb + 1], in_=el[:],
                                                 axis=mybir.AxisListType.X)

                    # FFN: h = gelu(aT2b @ w1) for all T, then y = h^T @ w2
                    with tc.tile_pool(name=f"moeh{l}", bufs=1) as hp_:
                        hT = hp_.tile([128, FB, T], BF16)
                        for fb in range(FB):
                            for tcH in range(2):
                                ph = psc.tile([128, 512], F32, tag="ps")
                                for cc in range(CC):
                                    nc.tensor.matmul(
                                        ph[:], w1sb[:, cc, fb * 128:(fb + 1) * 128],
                                        aT2b[:, cc, tcH * 512:(tcH + 1) * 512],
                                        start=(cc == 0), stop=(cc == CC - 1))
                                nc.scalar.activation(
                                    out=hT[:, fb, tcH * 512:(tcH + 1) * 512],
                                    in_=ph[:], func=AF.Gelu)

                        # y = hT^T @ w2, scaled by comb, -> ar_in (bf16)
                        for tb in range(TB):
                            for nch in range(2):
                                pyy = pbig.tile([128, 384], F32, tag="pb")
                                for fb in range(FB):
                                    nc.tensor.matmul(
                                        pyy[:],
                                        hT[:, fb, tb * 128:(tb + 1) * 128],
                                        w2sb[:, fb, nch * 384:(nch + 1) * 384],
                                        start=(fb == 0), stop=(fb == FB - 1))
                                ys = mt.tile([128, 384], BF16, tag="ys")
                                nc.vector.tensor_scalar_mul(out=ys[:], in0=pyy[:],
                                                            scalar1=comb[:, tb:tb + 1])
                                nc.sync.dma_start(
                                    ar_in[tb * 128:(tb + 1) * 128,
                                          nch * 384:(nch + 1) * 384], ys[:])

                # AllReduce expert contributions; X += sum
                nc.gpsimd.collective_compute(
                    "AllReduce", ALU.add,
                    replica_groups=[list(range(N_CORES))],
                    ins=[ar_in.opt()], outs=[ar_out.opt()])
                for tb in range(TB):
                    mo = small.tile([128, C], BF16, tag="mo")
                    nc.sync.dma_start(mo[:], ar_out[tb * 128:(tb + 1) * 128, :])
                    nc.vector.tensor_add(out=X[:, tb, :], in0=X[:, tb, :], in1=mo[:])


            # ======== final layer: only token block 7 feeds the output ========
            # (logits read x[:, -1:] only, so queries/MoE restricted to the
            # last 128 tokens; k/v still span all 1024 tokens)
            l = L - 1
            g1 = const.tile([128, C], F32, tag="g1", bufs=1)
            nc.sync.dma_start(g1[:], ln1_g[l])
            g2 = const.tile([128, C], F32, tag="g2", bufs=1)
            nc.sync.dma_start(g2[:], ln2_g[l])

            with tc.tile_pool(name="attnF", bufs=1) as ap:
                qkvw = ap.tile([128, CC, 3 * C], BF16)
                nc.sync.dma_start(
                    qkvw[:], qkv_wT[l].rearrange("(a p) d -> p a d", p=128))
                projw = ap.tile([128, CC, C], BF16)
                nc.sync.dma_start(
                    projw[:], proj_wT[l].rearrange("(a p) d -> p a d", p=128))

                kT = ap.tile([128, CC, T], BF16)
                vplus = ap.tile([128, TB, H, HD + 1], BF16)
                qT7 = ap.tile([128, CC, 128], BF16)   # last block, scaled 1/8

                with tc.tile_pool(name="aTF", bufs=1) as apT, \
                     tc.tile_pool(name="ptrAF", bufs=2, space="PSUM") as ptrA, \
                     tc.tile_pool(name="atmpAF", bufs=2) as at:
                    aT = apT.tile([128, CC, T], BF16)     # ln1(x)^T

                    for tb in range(TB):
                        a = at.tile([128, C], BF16, tag="lnout")
                        _ln_apply(nc, at, a[:], X[:, tb, :], g1[:], eps)
                        for cc in range(CC):
                            pt = ptrA.tile([128, 128], BF16, tag="ptb")
                            nc.tensor.transpose(out=pt[:],
                                                in_=a[:, cc * 128:(cc + 1) * 128],
                                                identity=identb[:])
                            nc.scalar.activation(out=aT[:, cc, tb * 128:(tb + 1) * 128],
                                                 in_=pt[:], func=AF.Copy)

                    # kT full; qT7 only for the last 128 tokens
                    for db in range(CC):
                        for tch in range(2):
                            ps = psc.tile([128, 512], F32, tag="ps")
                            for cc in range(CC):
                                nc.tensor.matmul(
                                    ps[:],
                                    qkvw[:, cc, C + db * 128:C + (db + 1) * 128],
                                    aT[:, cc, tch * 512:(tch + 1) * 512],
                                    start=(cc == 0), stop=(cc == CC - 1))
                            nc.scalar.activation(
                                out=kT[:, db, tch * 512:(tch + 1) * 512],
                                in_=ps[:], func=AF.Copy)
                    for db in range(CC):
                        ps = psc.tile([128, 512], F32, tag="ps")
                        for cc in range(CC):
                            nc.tensor.matmul(
                                ps[:, 0:128],
                                qkvw[:, cc, db * 128:(db + 1) * 128],
                                aT[:, cc, 7 * 128:T],
                                start=(cc == 0), stop=(cc == CC - 1))
                        nc.scalar.activation(out=qT7[:, db, :],
                                             in_=ps[:, 0:128], func=AF.Copy,
                                             scale=0.125)

                    for tb in range(TB):
                        nc.scalar.activation(out=vplus[:, tb, :, HD],
                                             in_=onesh[:], func=AF.Copy)
                    for tb in range(TB):
                        for nch in range(2):
                            pv = psc.tile([128, 384], F32, tag="ps")
                            for cc in range(CC):
                                nc.tensor.matmul(
                                    pv[:], aT[:, cc, tb * 128:(tb + 1) * 128],
                                    qkvw[:, cc, 2 * C + nch * 384:
                                         2 * C + (nch + 1) * 384],
                                    start=(cc == 0), stop=(cc == CC - 1))
                            dstv = vplus[:, tb, nch * 6:(nch + 1) * 6, 0:HD]
                            nc.vector.tensor_copy(out=dstv, in_=pv[:].rearrange(
                                "p (a b) -> p a b", b=HD))

                with tc.tile_pool(name="attBF", bufs=1) as bp, \
                     tc.tile_pool(name="pbigAF", bufs=2, space="PSUM") as pbigA, \
                     tc.tile_pool(name="atmpBF", bufs=2) as bt:
                    attT7 = bp.tile([128, CC, 128], BF16)

                    for h in range(H):
                        hc, hp = h // 2, (h % 2) * HD
                        pa = pav.tile([HD + 1, 128], F32, tag="pv")
                        for kb in range(TB):
                            ps = psc.tile([128, 128], F32, tag="ps")
                            nc.tensor.matmul(
                                ps[:], kT[hp:hp + HD, hc, kb * 128:(kb + 1) * 128],
                                qT7[hp:hp + HD, hc, :],
                                start=True, stop=True)
                            es = bt.tile([128, 128], BF16, tag="es")
                            if kb == TB - 1:  # diagonal block
                                ms = bt.tile([128, 128], F32, tag="ms")
                                nc.vector.tensor_tensor(
                                    out=ms[:], in0=ps[:],
                                    in1=cmask[:, 0, 0:128], op=ALU.add)
                                nc.scalar.activation(out=es[:], in_=ms[:],
                                                     func=AF.Exp)
                            else:
                                nc.scalar.activation(out=es[:], in_=ps[:],
                                                     func=AF.Exp)
                            nc.tensor.matmul(pa[:], vplus[:, kb, h, :], es[:],
                                             start=(kb == 0), stop=(kb == TB - 1))
                        sb = bt.tile([1, 128], F32R, tag="sb")
                        with nc.allow_low_precision(reason="f32r for PE bcast"):
                            nc.scalar.activation(out=sb[:], in_=pa[HD:HD + 1, :],
                                                 func=AF.Copy)
                        pb = psc.tile([128, 128], F32, tag="ps")
                        nc.tensor.matmul(pb[:HD, :], ones64[:], sb[:],
                                         start=True, stop=True)
                        rsb = bt.tile([HD, 128], F32, tag="rsb")
                        nc.vector.reciprocal(out=rsb[:], in_=pb[:HD, :])
                        nc.vector.tensor_tensor(
                            out=attT7[hp:hp + HD, hc, :],
                            in0=pa[:HD, :], in1=rsb[:], op=ALU.mult)

                    # proj for the last block; replicated, so straight X add
                    for nch in range(2):
                        py = pbigA.tile([128, 384], F32, tag="pb")
                        for cc in range(CC):
                            nc.tensor.matmul(
                                py[:], attT7[:, cc, :],
                                projw[:, cc, nch * 384:(nch + 1) * 384],
                                start=(cc == 0), stop=(cc == CC - 1))
                        nc.vector.tensor_add(
                            out=X[:, TB - 1, nch * 384:(nch + 1) * 384],
                            in0=X[:, TB - 1, nch * 384:(nch + 1) * 384], in1=py[:])

            # ---- thin MoE: only the last 128 tokens ----
            with tc.tile_pool(name="moeF", bufs=1) as mp, \
                 tc.tile_pool(name="mtmpF", bufs=2) as mt:
                w1sb = mp.tile([128, CC, F], BF16)
                nc.sync.dma_start(
                    w1sb[:], w1T[l].rearrange("(a p) d -> p a d", p=128))
                w2sb = mp.tile([128, FB, C], BF16)
                nc.sync.dma_start(
                    w2sb[:], w2T[l].rearrange("(a p) d -> p a d", p=128))

                aT27 = mp.tile([128, CC, 128], F32)
                aT27b = mp.tile([128, CC, 128], BF16)
                comb7 = mp.tile([128, 1], F32)

                with tc.tile_pool(name="moegF", bufs=1) as gp, \
                     tc.tile_pool(name="ptrMF", bufs=2, space="PSUM") as ptrM:
                    a = mt.tile([128, C], F32, tag="lnout2")
                    _ln_apply(nc, mt, a[:], X[:, TB - 1, :], g2[:], eps)
                    for cc in range(CC):
                        pt = ptrM.tile([128, 128], F32)
                        nc.tensor.transpose(out=pt[:],
                                            in_=a[:, cc * 128:(cc + 1) * 128],
                                            identity=ident[:])
                        nc.scalar.activation(out=aT27[:, cc, :],
                                             in_=pt[:], func=AF.Copy)
                        nc.vector.tensor_copy(out=aT27b[:, cc, :], in_=pt[:])

                    gwt = const.tile([128, CC, E], F32, tag="gw", bufs=1)
                    nc.sync.dma_start(gwt[:],
                                      gate_wT[l].rearrange("(a b) e -> b a e", b=128))
                    pg = pav.tile([128, E], F32, tag="pv")
                    for cc in range(CC):
                        nc.tensor.matmul(pg[:], aT27[:, cc, :], gwt[:, cc, :],
                                         start=(cc == 0), stop=(cc == CC - 1))
                    lg = mt.tile([128, E], F32, tag="lg")
                    nc.vector.tensor_copy(out=lg[:], in_=pg[:])
                    m8 = mt.tile([128, 8], F32, tag="m8")
                    nc.vector.max(out=m8[:], in_=lg[:])
                    nv0 = mt.tile([128, 1], F32, tag="nv0")
                    nc.vector.tensor_scalar_mul(out=nv0[:], in0=m8[:, 0:1],
                                                scalar1=-1.0)
                    el = mt.tile([128, E], F32, tag="el")
                    nc.scalar.activation(out=el[:], in_=lg[:], func=AF.Exp,
                                         bias=nv0[:])
                    e1 = mt.tile([128, 1], F32, tag="e1")
                    nc.scalar.activation(out=e1[:], in_=m8[:, 1:2], func=AF.Exp,
                                         bias=nv0[:])
                    nc.vector.tensor_scalar_add(out=e1[:], in0=e1[:], scalar1=1.0)
                    nc.vector.reciprocal(out=e1[:], in_=e1[:])
                    msk = mt.tile([128, E], F32, tag="msk")
                    nc.vector.tensor_scalar(out=msk[:], in0=lg[:],
                                            scalar1=m8[:, 1:2], scalar2=None,
                                            op0=ALU.is_ge)
                    nc.vector.tensor_tensor(out=el[:], in0=el[:], in1=msk[:],
                                            op=ALU.mult)
                    nc.vector.tensor_scalar_mul(out=el[:], in0=el[:], scalar1=e1[:])
                    nc.vector.tensor_tensor(out=el[:], in0=el[:], in1=evt[:],
                                            op=ALU.mult)
                    nc.vector.reduce_sum(out=comb7[:, 0:1], in_=el[:],
                                         axis=mybir.AxisListType.X)

                with tc.tile_pool(name="moehF", bufs=1) as hp_, \
                     tc.tile_pool(name="pbigMF", bufs=2, space="PSUM") as pbigM:
                    hT7 = hp_.tile([128, FB, 128], BF16)
                    for fb in range(FB):
                        ph = psc.tile([128, 128], F32, tag="ps")
                        for cc in range(CC):
                            nc.tensor.matmul(
                                ph[:], w1sb[:, cc, fb * 128:(fb + 1) * 128],
                                aT27b[:, cc, :],
                                start=(cc == 0), stop=(cc == CC - 1))
                        nc.scalar.activation(out=hT7[:, fb, :],
                                             in_=ph[:], func=AF.Gelu)

                    for nch in range(2):
                        pyy = pbigM.tile([128, 384], F32, tag="pb")
                        for fb in range(FB):
                            nc.tensor.matmul(
                                pyy[:], hT7[:, fb, :],
                                w2sb[:, fb, nch * 384:(nch + 1) * 384],
                                start=(fb == 0), stop=(fb == FB - 1))
                        ys = mt.tile([128, 384], BF16, tag="ys")
                        nc.vector.tensor_scalar_mul(out=ys[:], in0=pyy[:],
                                                    scalar1=comb7[:, 0:1])
                        nc.sync.dma_start(
                            ar_in[7 * 128:T, nch * 384:(nch + 1) * 384], ys[:])
                    nc.gpsimd.collective_compute(
                        "AllReduce", ALU.add,
                        replica_groups=[list(range(N_CORES))],
                        ins=[ar_in[7 * 128:T, :].opt()],
                        outs=[aro[2][0:128, :].opt()])
                    mo = small.tile([128, C], BF16, tag="mo")
                    nc.sync.dma_start(mo[:], aro[2][0:128, :])
                    nc.vector.tensor_add(out=X[:, TB - 1, :], in0=X[:, TB - 1, :],
                                         in1=mo[:])

            # ======== final LN (last token) + lm_head shard ========
            with tc.tile_pool(name="lmtail", bufs=1) as lt, \
                 tc.tile_pool(name="ptrL", bufs=2, space="PSUM") as ptrL, \
                 tc.tile_pool(name="wld", bufs=10) as wld:
                gf = lt.tile([1, C], F32, tag="gf")
                nc.sync.dma_start(gf[:], lnf_g[:])
                xrow = lt.tile([1, C], F32, tag="xrow")
                nc.sync.dma_start(xrow[:], X[127:128, TB - 1, :])
                xl = lt.tile([1, C], BF16, tag="xl")
                _ln_apply(nc, lt, xl[:1, :], xrow[:1, :], gf[:1, :], eps, rows=1)
                xlT = lt.tile([128, CC, 1], BF16, tag="xlT")
                for cc in range(CC):
                    pt = ptrL.tile([128, 128], BF16, tag="ptb")
                    nc.tensor.transpose(out=pt[:, 0:1],
                                        in_=xl[0:1, cc * 128:(cc + 1) * 128],
                                        identity=identb[0:1, 0:1])
                    nc.scalar.activation(out=xlT[:, cc, 0:1], in_=pt[:, 0:1], func=AF.Copy)
                nvc = VS // 512 + (1 if VS % 512 else 0)   # 13 chunks (last 140)
                lo = lt.tile([1, VS], F32, tag="lo")
                for vc in range(nvc):
                    w = min(512, VS - vc * 512)
                    pl = pav.tile([1, 512], F32, tag="pv")
                    for cc in range(CC):
                        wt_ = wld.tile([128, 512], BF16, tag="wte_t")
                        nc.gpsimd.dma_start(wt_[:, :w],
                                            wteT[cc * 128:(cc + 1) * 128,
                                                 vc * 512:vc * 512 + w])
                        nc.tensor.matmul(pl[:, :w], xlT[:, cc, 0:1], wt_[:, :w],
                                         start=(cc == 0), stop=(cc == CC - 1))
                    nc.vector.tensor_copy(out=lo[:, vc * 512:vc * 512 + w], in_=pl[:, :w])
                nc.sync.dma_start(out[:], lo[:])

    orig = nc.to_json_bytes
    nc.to_json_bytes = lambda: _legalize_bir_json(orig())
    return nc


_NC_CACHE = None


def kernel(**inputs):
    global _NC_CACHE
    idx = np.asarray(inputs["idx"]).astype(np.int32)
    wte = np.ascontiguousarray(np.asarray(inputs["wte"], dtype=np.float32))
    wpe = np.ascontiguousarray(np.asarray(inputs["wpe"], dtype=np.float32))
    ln1_g = np.asarray(inputs["ln1_g"], dtype=np.float32)
    qkv_w = np.asarray(inputs["qkv_w"], dtype=np.float32)
    proj_w = np.asarray(inputs["proj_w"], dtype=np.float32)
    ln2_g = np.asarray(inputs["ln2_g"], dtype=np.float32)
    gate_w = np.asarray(inputs["gate_w"], dtype=np.float32)
    w1 = np.asarray(inputs["w1"], dtype=np.float32)
    w2 = np.asarray(inputs["w2"], dtype=np.float32)
    lnf_g = np.asarray(inputs["lnf_g"], dtype=np.float32)

    qkv_wT = np.ascontiguousarray(qkv_w.transpose(0, 2, 1)).astype(BFNP)
    proj_wT = np.ascontiguousarray(proj_w.transpose(0, 2, 1)).astype(BFNP)
    gate_wT = np.ascontiguousarray(gate_w.transpose(0, 2, 1))
    ln1_rep = np.ascontiguousarray(np.broadcast_to(ln1_g[:, None, :], (L, 128, C)))
    ln2_rep = np.ascontiguousarray(np.broadcast_to(ln2_g[:, None, :], (L, 128, C)))
    wteT_full = np.zeros((C, N_CORES * VS), np.float32)
    wteT_full[:, :V] = wte.T
    wteT_full = wteT_full.astype(BFNP)

    if _NC_CACHE is None:
        _NC_CACHE = build_program()
    nc = _NC_CACHE

    # layer-0 attention is head-sharded: 2 slot-heads per core
    SLOTH = [[0, 1], [2, 3], [4, 5], [6, 7], [8, -1], [9, -1], [10, -1], [11, -1]]
    qkv_w0T = np.ascontiguousarray(qkv_w[0].T)    # [C, 3C] fp32
    proj_w0T = np.ascontiguousarray(proj_w[0].T)  # [C, C] fp32

    in_maps = []
    for c in range(N_CORES):
        ev = np.zeros((128, E), np.float32)
        ev[:, c] = 1.0
        aqkv_c = np.zeros((C, 384), np.float32)
        aproj_c = np.zeros((128, C), np.float32)
        for s, h in enumerate(SLOTH[c]):
            if h >= 0:
                aqkv_c[:, s * 64:(s + 1) * 64] = qkv_w0T[:, h * 64:(h + 1) * 64]
                aqkv_c[:, 128 + s * 64:128 + (s + 1) * 64] = \
                    qkv_w0T[:, C + h * 64:C + (h + 1) * 64]
                aqkv_c[:, 256 + s * 64:256 + (s + 1) * 64] = \
                    qkv_w0T[:, 2 * C + h * 64:2 * C + (h + 1) * 64]
                aproj_c[s * 64:(s + 1) * 64, :] = proj_w0T[h * 64:(h + 1) * 64, :]
        in_maps.append({
            "aqkv": aqkv_c.astype(BFNP),
            "aproj": aproj_c.astype(BFNP),
            "idx": idx,
            "wte": wte,
            "wpe": wpe,
            "qkv_wT": qkv_wT,
            "proj_wT": proj_wT,
            "gate_wT": gate_wT,
            "ln1_g": ln1_rep,
            "ln2_g": ln2_rep,
            "lnf_g": lnf_g[None, :],
            "evec": ev,
            "w1T": np.ascontiguousarray(w1[:, c].transpose(0, 2, 1)).astype(BFNP),
            "w2T": np.ascontiguousarray(w2[:, c].transpose(0, 2, 1)).astype(BFNP),
            "wteT": np.ascontiguousarray(wteT_full[:, c * VS:(c + 1) * VS]),
        })

    res = run_bass_kernel_spmd(nc, in_maps, list(range(N_CORES)))
    kernel.last_result = res
    logits = np.concatenate([res.results[c]["out"][0] for c in range(N_CORES)])
    return logits[:V].reshape(1, 1, V).astype(np.float32)


# revision 15
# speedup vs baseline: 4.9926x; 1.0242x over previous
"""MoE-GPT forward on 8 Trainium2 NeuronCores.

Sharding: residual stream replicated on all cores; attention replicated
(transposed-score layout, causal-skipped blocks); MoE expert-parallel
(core c owns expert c, dense over tokens, combine via AllReduce);
lm_head vocab-sharded (8 x 6284 columns), concatenated on host.

v2: bf16 matmul operands (weights cast on host, activations cast on
device) except the gating path which stays exact fp32 so top-2 routing
matches the reference; all per-layer weights resident in SBUF (no
weight DMA inside matmul loops); bf16 AllReduce; Rsqrt layernorm.
"""

import json
import numpy as np
import ml_dtypes

import concourse.bass as bass
import concourse.mybir as mybir
import concourse.tile as tile
from concourse.bass_utils import run_bass_kernel_spmd
from concourse.masks import make_identity

AF = mybir.ActivationFunctionType
ALU = mybir.AluOpType
F32 = mybir.dt.float32
F32R = mybir.dt.float32r
BF16 = mybir.dt.bfloat16
I32 = mybir.dt.int32

L, C, H, E, K, V, T = 2, 768, 12, 8, 2, 50257, 1024
HD = C // H          # 64
F = 4 * C            # 3072
N_CORES = 8
VS = 6284            # vocab shard per core (8*6284 = 50272 >= 50257)
CC = C // 128        # 6 c-chunks
TB = T // 128        # 8 token blocks
FB = F // 128        # 24 f blocks
NEG = -1.0e30
BFNP = ml_dtypes.bfloat16


def _legalize_bir_json(bir_bytes):
    """This walrus build accepts at most ONE sync wait per instruction;
    split extras onto standalone NoOps on the same engine."""
    m = json.loads(bir_bytes)
    for f in m["functions"]:
        for bb in f["blocks"]:
            out = []
            for inst in bb["instructions"]:
                si = inst.get("sync_info")
                if si:
                    waits = si.get("on_wait") or []
                    if len(waits) > 1:
                        imm = [w for w in waits if w.get("wait_reg") is None]
                        reg = [w for w in waits if w.get("wait_reg") is not None]
                        keep = reg if reg else [imm[-1]]
                        move = imm if reg else imm[:-1]
                        for j, w in enumerate(move):
                            out.append({
                                "debug": inst.get("debug", 0),
                                "engine": inst["engine"],
                                "ins": [], "outs": [],
                                "name": f"{inst['name']}-lw{j}",
                                "opcode": "NoOp",
                                "sync_info": {"on_wait": [w], "on_update": []},
                            })
                        si["on_wait"] = keep
                out.append(inst)
            bb["instructions"] = out
    return json.dumps(m).encode()


def _ln_apply(nc, pool, out_ap, in_ap, g_ap, eps_tile, rows=128):
    """LayerNorm rows of in_ap [rows, C] -> out_ap, gamma g_ap [rows, C]."""
    stats = pool.tile([128, 3, 6], F32, tag="ln_stats")
    mv = pool.tile([128, 2], F32, tag="ln_mv")
    xg = in_ap.rearrange("p (a b) -> p a b", b=256)
    for sg in range(3):
        nc.vector.bn_stats(out=stats[:rows, sg, :], in_=xg[:, sg, :])
    nc.vector.bn_aggr(out=mv[:rows, :], in_=stats[:rows, :, :])
    mean = mv[:rows, 0:1]
    rstd = pool.tile([128, 1], F32, tag="ln_rstd")
    nc.scalar.activation(out=rstd[:rows, :], in_=mv[:rows, 1:2],
                         func=AF.Sqrt, bias=eps_tile[:rows, :])
    nc.vector.reciprocal(out=rstd[:rows, :], in_=rstd[:rows, :])
    tmp = pool.tile([128, C], F32, tag="ln_tmp")
    nc.vector.tensor_scalar(out=tmp[:rows, :], in0=in_ap,
                            scalar1=mean, scalar2=rstd[:rows, :],
                            op0=ALU.subtract, op1=ALU.mult)
    nc.vector.tensor_tensor(out=out_ap, in0=tmp[:rows, :], in1=g_ap,
                            op=ALU.mult)


def build_program():
    nc = bass.Bass()
    nc._allow_low_precision_reason = "bf16/f32r matmul inputs are intentional"

    # ---- DRAM parameters ----
    idx = nc.declare_dram_parameter("idx", [1, T], I32, isOutput=False)
    wte = nc.declare_dram_parameter("wte", [V, C], F32, isOutput=False)
    wpe = nc.declare_dram_parameter("wpe", [T, C], F32, isOutput=False)
    qkv_wT = nc.declare_dram_parameter("qkv_wT", [L, C, 3 * C], BF16, isOutput=False)
    aqkv = nc.declare_dram_parameter("aqkv", [C, 384], BF16, isOutput=False)
    aproj = nc.declare_dram_parameter("aproj", [128, C], BF16, isOutput=False)
    proj_wT = nc.declare_dram_parameter("proj_wT", [L, C, C], BF16, isOutput=False)
    gate_wT = nc.declare_dram_parameter("gate_wT", [L, C, E], F32, isOutput=False)
    ln1_g = nc.declare_dram_parameter("ln1_g", [L, 128, C], F32, isOutput=False)
    ln2_g = nc.declare_dram_parameter("ln2_g", [L, 128, C], F32, isOutput=False)
    lnf_g = nc.declare_dram_parameter("lnf_g", [1, C], F32, isOutput=False)
    evec = nc.declare_dram_parameter("evec", [128, E], F32, isOutput=False)
    w1T = nc.declare_dram_parameter("w1T", [L, C, F], BF16, isOutput=False)
    w2T = nc.declare_dram_parameter("w2T", [L, F, C], BF16, isOutput=False)
    wteT = nc.declare_dram_parameter("wteT", [C, VS], BF16, isOutput=False)
    out = nc.declare_dram_parameter("out", [1, VS], F32, isOutput=True)

    with tile.TileContext(nc) as tc:
        with tc.tile_pool(name="const", bufs=1) as const, \
             tc.tile_pool(name="dram", bufs=1, space="DRAM") as dram, \
             tc.tile_pool(name="xp", bufs=1) as xp, \
             tc.tile_pool(name="small", bufs=2) as small, \
             tc.tile_pool(name="psc", bufs=2, space="PSUM") as psc, \
             tc.tile_pool(name="pav", bufs=2, space="PSUM") as pav, \
             tc.tile_pool(name="pbig", bufs=2, space="PSUM") as pbig:

            ident = const.tile([128, 128], F32)
            make_identity(nc, ident)
            identb = const.tile([128, 128], BF16)
            nc.vector.tensor_copy(out=identb[:], in_=ident[:])
            eps = const.tile([128, 1], F32)
            nc.vector.memset(eps[:], 1e-5)
            evt = const.tile([128, E], F32)
            nc.sync.dma_start(evt[:], evec[:])
            # causal masks for the 4 diagonal sub-block offsets:
            # mask[rel][p, qf] = 0 if qf - rel*128 - p >= 0 else -1e30
            ones64f = const.tile([1, HD], F32)
            nc.vector.memset(ones64f[:], 1.0)
            ones64 = const.tile([1, HD], F32R)
            nc.scalar.activation(out=ones64[:], in_=ones64f[:], func=AF.Copy)
            onesh = const.tile([128, H], BF16)
            nc.vector.memset(onesh[:], 1.0)
            cmask = const.tile([128, 4, 512], F32)
            nc.vector.memset(cmask[:], 0.0)
            for rel in range(4):
                nc.gpsimd.affine_select(
                    out=cmask[:, rel, :], in_=cmask[:, rel, :],
                    pattern=[[1, 512]], base=-rel * 128,
                    channel_multiplier=-1,
                    compare_op=ALU.is_ge, fill=NEG)

            # Residual stream, replicated: X[p, tb, c], token = tb*128+p
            X = xp.tile([128, TB, C], F32)

            # ---- embedding: X = wte[idx] + wpe ----
            for tb in range(TB):
                it = small.tile([128, 1], I32, tag="idx")
                nc.sync.dma_start(it[:], idx[0:1, tb * 128:(tb + 1) * 128]
                                  .rearrange("a b -> b a"))
                emb = small.tile([128, C], F32, tag="emb")
                nc.gpsimd.indirect_dma_start(
                    out=emb[:], out_offset=None, in_=wte[:, :],
                    in_offset=bass.IndirectOffsetOnAxis(ap=it[:, :1], axis=0))
                pe = small.tile([128, C], F32, tag="pe")
                nc.sync.dma_start(pe[:], wpe[tb * 128:(tb + 1) * 128, :])
                nc.gpsimd.tensor_add(out=X[:, tb, :], in0=emb[:], in1=pe[:])

            ar_in = dram.tile([T, C], BF16)
            ar_out = dram.tile([T, C], BF16)

            for l in range(L - 1):
                g1 = const.tile([128, C], F32, tag="g1", bufs=1)
                nc.sync.dma_start(g1[:], ln1_g[l])
                g2 = const.tile([128, C], F32, tag="g2", bufs=1)
                nc.sync.dma_start(g2[:], ln2_g[l])

                # ======== attention (head-sharded: 2 slot-heads per core,
                # partial proj combined via AllReduce) ========
                with tc.tile_pool(name=f"attn{l}", bufs=1) as ap:
                    aqkvw = ap.tile([128, CC, 384], BF16)
                    nc.sync.dma_start(
                        aqkvw[:], aqkv.rearrange("(a p) d -> p a d", p=128))
                    aprojw = ap.tile([128, C], BF16)
                    nc.sync.dma_start(aprojw[:], aproj[:, :])

                    qTs = ap.tile([128, T], BF16)     # scaled by 1/8
                    kTs = ap.tile([128, T], BF16)
                    vplus = ap.tile([128, TB, 2, HD + 1], BF16)

                    with tc.tile_pool(name=f"aT{l}", bufs=1) as apT, \
                         tc.tile_pool(name=f"ptrA{l}", bufs=2, space="PSUM") as ptrA, \
                         tc.tile_pool(name=f"atmpA{l}", bufs=2) as at:
                        aT = apT.tile([128, CC, T], BF16)     # ln1(x)^T

                        # ln1 (bf16 out) + transpose -> aT
                        for tb in range(TB):
                            a = at.tile([128, C], BF16, tag="lnout")
                            _ln_apply(nc, at, a[:], X[:, tb, :], g1[:], eps)
                            for cc in range(CC):
                                pt = ptrA.tile([128, 128], BF16, tag="ptb")
                                nc.tensor.transpose(out=pt[:],
                                                    in_=a[:, cc * 128:(cc + 1) * 128],
                                                    identity=identb[:])
                                nc.scalar.activation(out=aT[:, cc, tb * 128:(tb + 1) * 128],
                                                     in_=pt[:], func=AF.Copy)

                        # my 2 slot-heads' q, k : [128 dims, T]
                        for half, dst, scl in ((0, qTs, 0.125), (1, kTs, 1.0)):
                            for tch in range(2):
                                ps = psc.tile([128, 512], F32, tag="ps")
                                for cc in range(CC):
                                    nc.tensor.matmul(
                                        ps[:],
                                        aqkvw[:, cc, half * 128:(half + 1) * 128],
                                        aT[:, cc, tch * 512:(tch + 1) * 512],
                                        start=(cc == 0), stop=(cc == CC - 1))
                                nc.scalar.activation(
                                    out=dst[:, tch * 512:(tch + 1) * 512],
                                    in_=ps[:], func=AF.Copy, scale=scl)

                        # v token-major for my 2 slot-heads
                        for tb in range(TB):
                            nc.scalar.activation(out=vplus[:, tb, :, HD],
                                                 in_=onesh[:, 0:2], func=AF.Copy)
                        for tb in range(TB):
                            pv = psc.tile([128, 128], F32, tag="ps")
                            for cc in range(CC):
                                nc.tensor.matmul(
                                    pv[:], aT[:, cc, tb * 128:(tb + 1) * 128],
                                    aqkvw[:, cc, 256:384],
                                    start=(cc == 0), stop=(cc == CC - 1))
                            dstv = vplus[:, tb, :, 0:HD]
                            nc.vector.tensor_copy(out=dstv, in_=pv[:].rearrange(
                                "p (a b) -> p a b", b=HD))

                    with tc.tile_pool(name=f"attB{l}", bufs=1) as bp, \
                         tc.tile_pool(name=f"pbigA{l}", bufs=2, space="PSUM") as pbigA, \
                         tc.tile_pool(name=f"atmpB{l}", bufs=2) as bt:
                        attTs = bp.tile([128, T], BF16)

                        # scores^T + exp + av^T; token-half qc outer so the
                        # half's proj partial + AllReduce launch overlap the
                        # next half's score compute
                        for qc in range(2):
                            for slot in range(2):
                                hp = slot * HD
                                pa = pav.tile([HD + 1, 512], F32, tag="pv")
                                nkb = 4 * (qc + 1)
                                for kb in range(nkb):
                                    ps = psc.tile([128, 512], F32, tag="ps")
                                    nc.tensor.matmul(
                                        ps[:], kTs[hp:hp + HD, kb * 128:(kb + 1) * 128],
                                        qTs[hp:hp + HD, qc * 512:(qc + 1) * 512],
                                        start=True, stop=True)
                                    es = bt.tile([128, 512], BF16, tag="es")
                                    if kb >= 4 * qc:  # partial-causal block
                                        ms = bt.tile([128, 512], F32, tag="ms")
                                        nc.vector.tensor_tensor(
                                            out=ms[:], in0=ps[:],
                                            in1=cmask[:, kb - 4 * qc, :], op=ALU.add)
                                        nc.scalar.activation(out=es[:], in_=ms[:],
                                                             func=AF.Exp)
                                    else:
                                        nc.scalar.activation(out=es[:], in_=ps[:],
                                                             func=AF.Exp)
                                    nc.tensor.matmul(pa[:], vplus[:, kb, slot, :], es[:],
                                                     start=(kb == 0), stop=(kb == nkb - 1))
                                sb = bt.tile([1, 512], F32R, tag="sb")
                                with nc.allow_low_precision(reason="f32r for PE bcast"):
                                    nc.scalar.activation(out=sb[:], in_=pa[HD:HD + 1, :],
                                                         func=AF.Copy)
                                pb = psc.tile([128, 512], F32, tag="ps")
                                nc.tensor.matmul(pb[:HD, :], ones64[:], sb[:],
                                                 start=True, stop=True)
                                rsb = bt.tile([HD, 512], F32, tag="rsb")
                                nc.vector.reciprocal(out=rsb[:], in_=pb[:HD, :])
                                nc.vector.tensor_tensor(
                                    out=attTs[hp:hp + HD, qc * 512:(qc + 1) * 512],
                                    in0=pa[:HD, :], in1=rsb[:],
                                    op=ALU.mult)

                            for tq in range(4):
                                tb = qc * 4 + tq
                                for nch in range(2):
                                    py = pbigA.tile([128, 384], F32, tag="pb")
                                    nc.tensor.matmul(
                                        py[:], attTs[:, tb * 128:(tb + 1) * 128],
                                        aprojw[:, nch * 384:(nch + 1) * 384],
                                        start=True, stop=True)
                                    ya = bt.tile([128, 384], BF16, tag="ya")
                                    nc.vector.tensor_copy(out=ya[:], in_=py[:])
                                    nc.sync.dma_start(
                                        arA_in[tb * 128:(tb + 1) * 128,
                                               nch * 384:(nch + 1) * 384], ya[:])
                                if tq % 2 == 1:
                                    ch = (qc * 4 + tq) // 2
                                    nc.gpsimd.collective_compute(
                                        "AllReduce", ALU.add,
                                        replica_groups=[list(range(N_CORES))],
                                        ins=[arA_in[ch * 256:(ch + 1) * 256, :].opt()],
                                        outs=[aroA[ch].opt()])

                # ======== MoE (dense, expert-parallel) ========
                with tc.tile_pool(name=f"moe{l}", bufs=1) as mp, \
                     tc.tile_pool(name=f"mtmp{l}", bufs=2) as mt:
                    # resident FFN weights (bf16)
                    w1sb = mp.tile([128, CC, F], BF16)
                    nc.sync.dma_start(
                        w1sb[:], w1T[l].rearrange("(a p) d -> p a d", p=128))
                    w2sb = mp.tile([128, FB, C], BF16)
                    nc.sync.dma_start(
                        w2sb[:], w2T[l].rearrange("(a p) d -> p a d", p=128))

                    aT2b = mp.tile([128, CC, T], BF16)    # bf16, for FFN
                    comb = mp.tile([128, TB], F32)        # my expert's weight

                    with tc.tile_pool(name=f"moeg{l}", bufs=1) as gp, \
                         tc.tile_pool(name=f"ptrM{l}", bufs=2, space="PSUM") as ptrM:
                        aT2 = gp.tile([128, CC, T], F32)  # exact, for gating
                        for tb in range(TB):
                            if tb % 2 == 0:
                                # that chunk's attention AllReduce has landed;
                                # fold it into X before using those rows
                                ch = tb // 2
                                for tq in range(2):
                                    mo = small.tile([128, C], BF16, tag="mo")
                                    nc.sync.dma_start(
                                        mo[:], aroA[ch][tq * 128:(tq + 1) * 128, :])
                                    nc.vector.tensor_add(out=X[:, tb + tq, :],
                                                         in0=X[:, tb + tq, :],
                                                         in1=mo[:])
                            a = mt.tile([128, C], F32, tag="lnout2")
                            _ln_apply(nc, mt, a[:], X[:, tb, :], g2[:], eps)
                            for cc in range(CC):
                                pt = ptrM.tile([128, 128], F32)
                                nc.tensor.transpose(out=pt[:],
                                                    in_=a[:, cc * 128:(cc + 1) * 128],
                                                    identity=ident[:])
                                nc.scalar.activation(out=aT2[:, cc, tb * 128:(tb + 1) * 128],
                                                     in_=pt[:], func=AF.Copy)
                                nc.vector.tensor_copy(out=aT2b[:, cc, tb * 128:(tb + 1) * 128],
                                                      in_=pt[:])

                        # gating (exact fp32) + combine weight for my expert
                        gwt = const.tile([128, CC, E], F32, tag="gw", bufs=1)
                        nc.sync.dma_start(gwt[:],
                                          gate_wT[l].rearrange("(a b) e -> b a e", b=128))
                        for tb in range(TB):
                            pg = pav.tile([128, E], F32, tag="pv")
                            for cc in range(CC):
                                nc.tensor.matmul(pg[:], aT2[:, cc, tb * 128:(tb + 1) * 128],
                                                 gwt[:, cc, :],
                                                 start=(cc == 0), stop=(cc == CC - 1))
                            lg = mt.tile([128, E], F32, tag="lg")
                            nc.vector.tensor_copy(out=lg[:], in_=pg[:])
                            m8 = mt.tile([128, 8], F32, tag="m8")
                            nc.vector.max(out=m8[:], in_=lg[:])
                            nv0 = mt.tile([128, 1], F32, tag="nv0")
                            nc.vector.tensor_scalar_mul(out=nv0[:], in0=m8[:, 0:1],
                                                        scalar1=-1.0)
                            el = mt.tile([128, E], F32, tag="el")
                            nc.scalar.activation(out=el[:], in_=lg[:], func=AF.Exp,
                                                 bias=nv0[:])
                            e1 = mt.tile([128, 1], F32, tag="e1")
                            nc.scalar.activation(out=e1[:], in_=m8[:, 1:2], func=AF.Exp,
                                                 bias=nv0[:])
                            nc.vector.tensor_scalar_add(out=e1[:], in0=e1[:], scalar1=1.0)
                            nc.vector.reciprocal(out=e1[:], in_=e1[:])
                            msk = mt.tile([128, E], F32, tag="msk")
                            nc.vector.tensor_scalar(out=msk[:], in0=lg[:],
                                                    scalar1=m8[:, 1:2], scalar2=None,
                                                    op0=ALU.is_ge)
                            nc.vector.tensor_tensor(out=el[:], in0=el[:], in1=msk[:],
                                                    op=ALU.mult)
                            nc.vector.tensor_scalar_mul(out=el[:], in0=el[:], scalar1=e1[:])
                            nc.vector.tensor_tensor(out=el[:], in0=el[:], in1=evt[:],
                                                    op=ALU.mult)
                            nc.vector.reduce_sum(out=comb[:, tb:tb + 1], in_=el[:],
                                                 axis=mybir.AxisListType.X)

                    # FFN: h = gelu(aT2b @ w1) for all T, then y = h^T @ w2
                    with tc.tile_pool(name=f"moeh{l}", bufs=1) as hp_:
                        hT = hp_.tile([128, FB, T], BF16)
                        for fb in range(FB):
                            for tcH in range(2):
                                ph = psc.tile([128, 512], F32, tag="ps")
                                for cc in range(CC):
                                    nc.tensor.matmul(
                                        ph[:], w1sb[:, cc, fb * 128:(fb + 1) * 128],
                                        aT2b[:, cc, tcH * 512:(tcH + 1) * 512],
                                        start=(cc == 0), stop=(cc == CC - 1))
                                nc.scalar.activation(
                                    out=hT[:, fb, tcH * 512:(tcH + 1) * 512],
                                    in_=ph[:], func=AF.Gelu)

                        # y = hT^T @ w2, scaled by comb, -> ar_in (bf16)
                        for tb in range(TB):
                            for nch in range(2):
                                pyy = pbig.tile([128, 384], F32, tag="pb")
                                for fb in range(FB):
                                    nc.tensor.matmul(
                                        pyy[:],
                                        hT[:, fb, tb * 128:(tb + 1) * 128],
                                        w2sb[:, fb, nch * 384:(nch + 1) * 384],
                                        start=(fb == 0), stop=(fb == FB - 1))
                                ys = mt.tile([128, 384], BF16, tag="ys")
                                nc.vector.tensor_scalar_mul(out=ys[:], in0=pyy[:],
                                                            scalar1=comb[:, tb:tb + 1])
                                nc.sync.dma_start(
                                    ar_in[tb * 128:(tb + 1) * 128,
                                          nch * 384:(nch + 1) * 384], ys[:])

                # AllReduce expert contributions; X += sum
                nc.gpsimd.collective_compute(
                    "AllReduce", ALU.add,
                    replica_groups=[list(range(N_CORES))],
                    ins=[ar_in.opt()], outs=[ar_out.opt()])
                for tb in range(TB):
                    mo = small.tile([128, C], BF16, tag="mo")
                    nc.sync.dma_start(mo[:], ar_out[tb * 128:(tb + 1) * 128, :])
                    nc.vector.tensor_add(out=X[:, tb, :], in0=X[:, tb, :], in1=mo[:])


            # ======== final layer: only token block 7 feeds the output ========
            # (logits read x[:, -1:] only, so queries/MoE restricted to the
            # last 128 tokens; k/v still span all 1024 tokens)
            l = L - 1
            g1 = const.tile([128, C], F32, tag="g1", bufs=1)
            nc.sync.dma_start(g1[:], ln1_g[l])
            g2 = const.tile([128, C], F32, tag="g2", bufs=1)
            nc.sync.dma_start(g2[:], ln2_g[l])

            with tc.tile_pool(name="attnF", bufs=1) as ap:
                qkvw = ap.tile([128, CC, 3 * C], BF16)
                nc.sync.dma_start(
                    qkvw[:], qkv_wT[l].rearrange("(a p) d -> p a d", p=128))
                projw = ap.tile([128, CC, C], BF16)
                nc.sync.dma_start(
                    projw[:], proj_wT[l].rearrange("(a p) d -> p a d", p=128))

                kT = ap.tile([128, CC, T], BF16)
                vplus = ap.tile([128, TB, H, HD + 1], BF16)
                qT7 = ap.tile([128, CC, 128], BF16)   # last block, scaled 1/8

                with tc.tile_pool(name="aTF", bufs=1) as apT, \
                     tc.tile_pool(name="ptrAF", bufs=2, space="PSUM") as ptrA, \
                     tc.tile_pool(name="atmpAF", bufs=2) as at:
                    aT = apT.tile([128, CC, T], BF16)     # ln1(x)^T

                    for tb in range(TB):
                        a = at.tile([128, C], BF16, tag="lnout")
                        _ln_apply(nc, at, a[:], X[:, tb, :], g1[:], eps)
                        for cc in range(CC):
                            pt = ptrA.tile([128, 128], BF16, tag="ptb")
                            nc.tensor.transpose(out=pt[:],
                                                in_=a[:, cc * 128:(cc + 1) * 128],
                                                identity=identb[:])
                            nc.scalar.activation(out=aT[:, cc, tb * 128:(tb + 1) * 128],
                                                 in_=pt[:], func=AF.Copy)

                    # kT full; qT7 only for the last 128 tokens
                    for db in range(CC):
                        for tch in range(2):
                            ps = psc.tile([128, 512], F32, tag="ps")
                            for cc in range(CC):
                                nc.tensor.matmul(
                                    ps[:],
                                    qkvw[:, cc, C + db * 128:C + (db + 1) * 128],
                                    aT[:, cc, tch * 512:(tch + 1) * 512],
                                    start=(cc == 0), stop=(cc == CC - 1))
                            nc.scalar.activation(
                                out=kT[:, db, tch * 512:(tch + 1) * 512],
                                in_=ps[:], func=AF.Copy)
                    for db in range(CC):
                        ps = psc.tile([128, 512], F32, tag="ps")
                        for cc in range(CC):
                            nc.tensor.matmul(
                                ps[:, 0:128],
                                qkvw[:, cc, db * 128:(db + 1) * 128],
                                aT[:, cc, 7 * 128:T],
                                start=(cc == 0), stop=(cc == CC - 1))
                        nc.scalar.activation(out=qT7[:, db, :],
                                             in_=ps[:, 0:128], func=AF.Copy,
                                             scale=0.125)

                    for tb in range(TB):
                        nc.scalar.activation(out=vplus[:, tb, :, HD],
                                             in_=onesh[:], func=AF.Copy)
                    for tb in range(TB):
                        for nch in range(2):
                            pv = psc.tile([128, 384], F32, tag="ps")
                            for cc in range(CC):
                                nc.tensor.matmul(
                                    pv[:], aT[:, cc, tb * 128:(tb + 1) * 128],
                                    qkvw[:, cc, 2 * C + nch * 384:
                                         2 * C + (nch + 1) * 384],
                                    start=(cc == 0), stop=(cc == CC - 1))
                            dstv = vplus[:, tb, nch * 6:(nch + 1) * 6, 0:HD]
                            nc.vector.tensor_copy(out=dstv, in_=pv[:].rearrange(
                                "p (a b) -> p a b", b=HD))

                with tc.tile_pool(name="attBF", bufs=1) as bp, \
                     tc.tile_pool(name="pbigAF", bufs=2, space="PSUM") as pbigA, \
                     tc.tile_pool(name="atmpBF", bufs=2) as bt:
                    attT7 = bp.tile([128, CC, 128], BF16)

                    for h in range(H):
                        hc, hp = h // 2, (h % 2) * HD
                        pa = pav.tile([HD + 1, 128], F32, tag="pv")
                        for kb in range(TB):
                            ps = psc.tile([128, 128], F32, tag="ps")
                            nc.tensor.matmul(
                                ps[:], kT[hp:hp + HD, hc, kb * 128:(kb + 1) * 128],
                                qT7[hp:hp + HD, hc, :],
                                start=True, stop=True)
                            es = bt.tile([128, 128], BF16, tag="es")
                            if kb == TB - 1:  # diagonal block
                                ms = bt.tile([128, 128], F32, tag="ms")
                                nc.vector.tensor_tensor(
                                    out=ms[:], in0=ps[:],
                                    in1=cmask[:, 0, 0:128], op=ALU.add)
                                nc.scalar.activation(out=es[:], in_=ms[:],
                                                     func=AF.Exp)
                            else:
                                nc.scalar.activation(out=es[:], in_=ps[:],
                                                     func=AF.Exp)
                            nc.tensor.matmul(pa[:], vplus[:, kb, h, :], es[:],
                                             start=(kb == 0), stop=(kb == TB - 1))
                        sb = bt.tile([1, 128], F32R, tag="sb")
                        with nc.allow_low_precision(reason="f32r for PE bcast"):
                            nc.scalar.activation(out=sb[:], in_=pa[HD:HD + 1, :],
                                                 func=AF.Copy)
                        pb = psc.tile([128, 128], F32, tag="ps")
                        nc.tensor.matmul(pb[:HD, :], ones64[:], sb[:],
                                         start=True, stop=True)
                        rsb = bt.tile([HD, 128], F32, tag="rsb")
                        nc.vector.reciprocal(out=rsb[:], in_=pb[:HD, :])
                        nc.vector.tensor_tensor(
                            out=attT7[hp:hp + HD, hc, :],
                            in0=pa[:HD, :], in1=rsb[:], op=ALU.mult)

                    # proj for the last block; replicated, so straight X add
                    for nch in range(2):
                        py = pbigA.tile([128, 384], F32, tag="pb")
                        for cc in range(CC):
                            nc.tensor.matmul(
                                py[:], attT7[:, cc, :],
                                projw[:, cc, nch * 384:(nch + 1) * 384],
                                start=(cc == 0), stop=(cc == CC - 1))
                        nc.vector.tensor_add(
                            out=X[:, TB - 1, nch * 384:(nch + 1) * 384],
                            in0=X[:, TB - 1, nch * 384:(nch + 1) * 384], in1=py[:])

            # prefetch the first 6 lm-head vocab chunks while the thin
            # MoE (and its AllReduces) run; the tail streams the rest
            wpre_ctx = tc.tile_pool(name="wpre", bufs=1)
            wpre = wpre_ctx.__enter__()
            wpreT = wpre.tile([128, 6, CC, 512], BF16)
            for vc in range(6):
                nc.sync.dma_start(
                    wpreT[:, vc], wteT[:, vc * 512:(vc + 1) * 512]
                    .rearrange("(a p) d -> p a d", p=128))

            # ---- thin MoE: only the last 128 tokens ----
            with tc.tile_pool(name="moeF", bufs=1) as mp, \
                 tc.tile_pool(name="mtmpF", bufs=2) as mt:
                w1sb = mp.tile([128, CC, F], BF16)
                nc.sync.dma_start(
                    w1sb[:], w1T[l].rearrange("(a p) d -> p a d", p=128))
                w2sb = mp.tile([128, FB, C], BF16)
                nc.sync.dma_start(
                    w2sb[:], w2T[l].rearrange("(a p) d -> p a d", p=128))

                aT27 = mp.tile([128, CC, 128], F32)
                aT27b = mp.tile([128, CC, 128], BF16)
                comb7 = mp.tile([128, 1], F32)

                with tc.tile_pool(name="moegF", bufs=1) as gp, \
                     tc.tile_pool(name="ptrMF", bufs=2, space="PSUM") as ptrM:
                    a = mt.tile([128, C], F32, tag="lnout2")
                    _ln_apply(nc, mt, a[:], X[:, TB - 1, :], g2[:], eps)
                    for cc in range(CC):
                        pt = ptrM.tile([128, 128], F32)
                        nc.tensor.transpose(out=pt[:],
                                            in_=a[:, cc * 128:(cc + 1) * 128],
                                            identity=ident[:])
                        nc.scalar.activation(out=aT27[:, cc, :],
                                             in_=pt[:], func=AF.Copy)
                        nc.vector.tensor_copy(out=aT27b[:, cc, :], in_=pt[:])

                    gwt = const.tile([128, CC, E], F32, tag="gw", bufs=1)
                    nc.sync.dma_start(gwt[:],
                                      gate_wT[l].rearrange("(a b) e -> b a e", b=128))
                    pg = pav.tile([128, E], F32, tag="pv")
                    for cc in range(CC):
                        nc.tensor.matmul(pg[:], aT27[:, cc, :], gwt[:, cc, :],
                                         start=(cc == 0), stop=(cc == CC - 1))
                    lg = mt.tile([128, E], F32, tag="lg")
                    nc.vector.tensor_copy(out=lg[:], in_=pg[:])
                    m8 = mt.tile([128, 8], F32, tag="m8")
                    nc.vector.max(out=m8[:], in_=lg[:])
                    nv0 = mt.tile([128, 1], F32, tag="nv0")
                    nc.vector.tensor_scalar_mul(out=nv0[:], in0=m8[:, 0:1],
                                                scalar1=-1.0)
                    el = mt.tile([128, E], F32, tag="el")
                    nc.scalar.activation(out=el[:], in_=lg[:], func=AF.Exp,
                                         bias=nv0[:])
                    e1 = mt.tile([128, 1], F32, tag="e1")
                    nc.scalar.activation(out=e1[:], in_=m8[:, 1:2], func=AF.Exp,
                                         bias=nv0[:])
                    nc.vector.tensor_scalar_add(out=e1[:], in0=e1[:], scalar1=1.0)
                    nc.vector.reciprocal(out=e1[:], in_=e1[:])
                    msk = mt.tile([128, E], F32, tag="msk")
                    nc.vector.tensor_scalar(out=msk[:], in0=lg[:],
                                            scalar1=m8[:, 1:2], scalar2=None,
                                            op0=ALU.is_ge)
                    nc.vector.tensor_tensor(out=el[:], in0=el[:], in1=msk[:],
                                            op=ALU.mult)
                    nc.vector.tensor_scalar_mul(out=el[:], in0=el[:], scalar1=e1[:])
                    nc.vector.tensor_tensor(out=el[:], in0=el[:], in1=evt[:],
                                            op=ALU.mult)
                    nc.vector.reduce_sum(out=comb7[:, 0:1], in_=el[:],
                                         axis=mybir.AxisListType.X)

                with tc.tile_pool(name="moehF", bufs=1) as hp_, \
                     tc.tile_pool(name="pbigMF", bufs=2, space="PSUM") as pbigM:
                    hT7 = hp_.tile([128, FB, 128], BF16)
                    for fb in range(FB):
                        ph = psc.tile([128, 128], F32, tag="ps")
                        for cc in range(CC):
                            nc.tensor.matmul(
                                ph[:], w1sb[:, cc, fb * 128:(fb + 1) * 128],
                                aT27b[:, cc, :],
                                start=(cc == 0), stop=(cc == CC - 1))
                        nc.scalar.activation(out=hT7[:, fb, :],
                                             in_=ph[:], func=AF.Gelu)

                    for nch in range(2):
                        pyy = pbigM.tile([128, 384], F32, tag="pb")
                        for fb in range(FB):
                            nc.tensor.matmul(
                                pyy[:], hT7[:, fb, :],
                                w2sb[:, fb, nch * 384:(nch + 1) * 384],
                                start=(fb == 0), stop=(fb == FB - 1))
                        ys = mt.tile([128, 384], BF16, tag="ys")
                        nc.vector.tensor_scalar_mul(out=ys[:], in0=pyy[:],
                                                    scalar1=comb7[:, 0:1])
                        nc.sync.dma_start(
                            ar_in[7 * 128:T, nch * 384:(nch + 1) * 384], ys[:])
                    nc.gpsimd.collective_compute(
                        "AllReduce", ALU.add,
                        replica_groups=[list(range(N_CORES))],
                        ins=[ar_in[7 * 128:T, :].opt()],
                        outs=[aro[2][0:128, :].opt()])
                    mo = small.tile([128, C], BF16, tag="mo")
                    nc.sync.dma_start(mo[:], aro[2][0:128, :])
                    nc.vector.tensor_add(out=X[:, TB - 1, :], in0=X[:, TB - 1, :],
                                         in1=mo[:])

            # ======== final LN (last token) + lm_head shard ========
            with tc.tile_pool(name="lmtail", bufs=1) as lt, \
                 tc.tile_pool(name="ptrL", bufs=2, space="PSUM") as ptrL, \
                 tc.tile_pool(name="wld", bufs=10) as wld:
                gf = lt.tile([1, C], F32, tag="gf")
                nc.sync.dma_start(gf[:], lnf_g[:])
                xrow = lt.tile([1, C], F32, tag="xrow")
                nc.sync.dma_start(xrow[:], X[127:128, TB - 1, :])
                xl = lt.tile([1, C], BF16, tag="xl")
                _ln_apply(nc, lt, xl[:1, :], xrow[:1, :], gf[:1, :], eps, rows=1)
                xlT = lt.tile([128, CC, 1], BF16, tag="xlT")
                for cc in range(CC):
                    pt = ptrL.tile([128, 128], BF16, tag="ptb")
                    nc.tensor.transpose(out=pt[:, 0:1],
                                        in_=xl[0:1, cc * 128:(cc + 1) * 128],
                                        identity=identb[0:1, 0:1])
                    nc.scalar.activation(out=xlT[:, cc, 0:1], in_=pt[:, 0:1], func=AF.Copy)
                nvc = VS // 512 + (1 if VS % 512 else 0)   # 13 chunks (last 140)
                lo = lt.tile([1, VS], F32, tag="lo")
                for vc in range(nvc):
                    w = min(512, VS - vc * 512)
                    pl = pav.tile([1, 512], F32, tag="pv")
                    for cc in range(CC):
                        if vc < 6:
                            wt_ap = wpreT[:, vc, cc, :w]
                        else:
                            wt_ = wld.tile([128, 512], BF16, tag="wte_t")
                            nc.gpsimd.dma_start(wt_[:, :w],
                                                wteT[cc * 128:(cc + 1) * 128,
                                                     vc * 512:vc * 512 + w])
                            wt_ap = wt_[:, :w]
                        nc.tensor.matmul(pl[:, :w], xlT[:, cc, 0:1], wt_ap,
                                         start=(cc == 0), stop=(cc == CC - 1))
                    nc.vector.tensor_copy(out=lo[:, vc * 512:vc * 512 + w], in_=pl[:, :w])
                nc.sync.dma_start(out[:], lo[:])
            wpre_ctx.__exit__(None, None, None)

    orig = nc.to_json_bytes
    nc.to_json_bytes = lambda: _legalize_bir_json(orig())
    return nc


_NC_CACHE = None


def kernel(**inputs):
    global _NC_CACHE
    idx = np.asarray(inputs["idx"]).astype(np.int32)
    wte = np.ascontiguousarray(np.asarray(inputs["wte"], dtype=np.float32))
    wpe = np.ascontiguousarray(np.asarray(inputs["wpe"], dtype=np.float32))
    ln1_g = np.asarray(inputs["ln1_g"], dtype=np.float32)
    qkv_w = np.asarray(inputs["qkv_w"], dtype=np.float32)
    proj_w = np.asarray(inputs["proj_w"], dtype=np.float32)
    ln2_g = np.asarray(inputs["ln2_g"], dtype=np.float32)
    gate_w = np.asarray(inputs["gate_w"], dtype=np.float32)
    w1 = np.asarray(inputs["w1"], dtype=np.float32)
    w2 = np.asarray(inputs["w2"], dtype=np.float32)
    lnf_g = np.asarray(inputs["lnf_g"], dtype=np.float32)

    qkv_wT = np.ascontiguousarray(qkv_w.transpose(0, 2, 1)).astype(BFNP)
    proj_wT = np.ascontiguousarray(proj_w.transpose(0, 2, 1)).astype(BFNP)
    gate_wT = np.ascontiguousarray(gate_w.transpose(0, 2, 1))
    ln1_rep = np.ascontiguousarray(np.broadcast_to(ln1_g[:, None, :], (L, 128, C)))
    ln2_rep = np.ascontiguousarray(np.broadcast_to(ln2_g[:, None, :], (L, 128, C)))
    wteT_full = np.zeros((C, N_CORES * VS), np.float32)
    wteT_full[:, :V] = wte.T
    wteT_full = wteT_full.astype(BFNP)

    if _NC_CACHE is None:
        _NC_CACHE = build_program()
    nc = _NC_CACHE

    # layer-0 attention is head-sharded: 2 slot-heads per core
    SLOTH = [[0, 1], [2, 3], [4, 5], [6, 7], [8, -1], [9, -1], [10, -1], [11, -1]]
    qkv_w0T = np.ascontiguousarray(qkv_w[0].T)    # [C, 3C] fp32
    proj_w0T = np.ascontiguousarray(proj_w[0].T)  # [C, C] fp32

    in_maps = []
    for c in range(N_CORES):
        ev = np.zeros((128, E), np.float32)
        ev[:, c] = 1.0
        aqkv_c = np.zeros((C, 384), np.float32)
        aproj_c = np.zeros((128, C), np.float32)
        for s, h in enumerate(SLOTH[c]):
            if h >= 0:
                aqkv_c[:, s * 64:(s + 1) * 64] = qkv_w0T[:, h * 64:(h + 1) * 64]
                aqkv_c[:, 128 + s * 64:128 + (s + 1) * 64] = \
                    qkv_w0T[:, C + h * 64:C + (h + 1) * 64]
                aqkv_c[:, 256 + s * 64:256 + (s + 1) * 64] = \
                    qkv_w0T[:, 2 * C + h * 64:2 * C + (h + 1) * 64]
                aproj_c[s * 64:(s + 1) * 64, :] = proj_w0T[h * 64:(h + 1) * 64, :]
        in_maps.append({
            "aqkv": aqkv_c.astype(BFNP),
            "aproj": aproj_c.astype(BFNP),
            "idx": idx,
            "wte": wte,
            "wpe": wpe,
            "qkv_wT": qkv_wT,
            "proj_wT": proj_wT,
            "gate_wT": gate_wT,
            "ln1_g": ln1_rep,
            "ln2_g": ln2_rep,
            "lnf_g": lnf_g[None, :],
            "evec": ev,
            "w1T": np.ascontiguousarray(w1[:, c].transpose(0, 2, 1)).astype(BFNP),
            "w2T": np.ascontiguousarray(w2[:, c].transpose(0, 2, 1)).astype(BFNP),
            "wteT": np.ascontiguousarray(wteT_full[:, c * VS:(c + 1) * VS]),
        })

    res = run_bass_kernel_spmd(nc, in_maps, list(range(N_CORES)))
    kernel.last_result = res
    logits = np.concatenate([res.results[c]["out"][0] for c in range(N_CORES)])
    return logits[:V].reshape(1, 1, V).astype(np.float32)
